# revision 5
# baseline (speedup 1.0000x reference)
"""DJMGNN (NNConv/GraphNorm GNN) Trainium2 kernel, 8-core SPMD. v2.

Sharding: nodes range-sharded N/8 per core, then PERMUTED within each shard so
every 128-node dst window holds <=512 edges (uniform 4 blocks/window, B=80
vs 98 unbalanced). Edges assigned to the core owning their dst node.

Per layer:
  - h table in DRAM; per-block indirect-DMA gather of h[src] (Pool engine).
    Layer 0 needs no table/gather: h0 = x@W is computed on HOST and shipped
    pre-gathered (hsrc0) plus the local shard (h0_own).
  - edge MLP on PE (attrT stationary, 2x512-col matmuls into one 2-bank PSUM
    tile) -> Act relu -> bf16 SBUF ring.
  - relu*h mult on DVE (bf16 2x) or Pool (layer 0), o-major broadcast AP.
  - one-hot scatter-matmul accumulating aggI[u,(o,i)] per 128-dst window.
  - interleaved node pass 1 at each window close: DVE strided i-reduce,
    root matmul (catT stationary), conv, stats via masked-ones matmul.
  - stats AllReduce; next-layer MLP runahead emitted before it to hide
    collective latency; batched pass 2; AllGather of the new h shard.
  - final transition + GraphNorm fused into layer-2 pass 2.
"""

import sys

if "/opt/trn_rl_repo" not in sys.path:
    sys.path.insert(0, "/opt/trn_rl_repo")

import numpy as np
import ml_dtypes

import concourse.bass as bass
import concourse.bacc as bacc
import concourse.mybir as mybir
import concourse.tile as tile

mdt = mybir.dt
AF = mybir.ActivationFunctionType
ALU = mybir.AluOpType

NCORES = 8
EPS = 1e-5
RUNA = 8  # next-layer MLP runahead blocks (hide stats-AllReduce latency)


# ---------------------------------------------------------------- host prep


def _balance_windows(deg, nwin, cap_n=128):
    """Assign local nodes to nwin windows, balancing edge load (greedy LPT)."""
    order = np.argsort(-deg, kind="stable")
    wload = np.zeros(nwin, dtype=np.int64)
    wn = np.zeros(nwin, dtype=np.int64)
    assign = np.full(deg.shape[0], -1, dtype=np.int64)
    for v in order:
        cand = np.where(wn < cap_n)[0]
        w = cand[np.argmin(wload[cand])]
        assign[v] = w
        wload[w] += deg[v]
        wn[w] += 1
    return assign, wload, wn


def prep_inputs(inputs):
    x = np.asarray(inputs["x"], np.float32)
    edge_attr = np.asarray(inputs["edge_attr"], np.float32)
    edge_index = np.asarray(inputs["edge_index"])
    N, IN = x.shape
    E, EA = edge_attr.shape
    H = np.asarray(inputs["init_W"]).shape[1]
    L = np.asarray(inputs["edge_mlp_W"]).shape[0]
    T = np.asarray(inputs["final_W"]).shape[1]
    shard = N // NCORES
    nwin = (shard + 127) // 128
    shard_pad = nwin * 128
    src = edge_index[0].astype(np.int64)
    dst = edge_index[1].astype(np.int64)
    owner = dst // shard
    dst_local = dst - owner * shard

    # per-core window assignment (node permutation)
    assigns, wns = [], []
    pos_of_node = np.empty((NCORES, shard), np.int64)  # node -> padded position
    node_at_pos = np.full((NCORES, shard_pad), -1, np.int64)
    cnt = np.zeros((NCORES, nwin), np.int64)
    for c in range(NCORES):
        dl = dst_local[owner == c]
        deg = np.bincount(dl, minlength=shard)
        assign, wload, wn = _balance_windows(deg, nwin)
        assigns.append(assign)
        wns.append(wn)
        cnt[c] = wload
        row_next = np.zeros(nwin, np.int64)
        for v in range(shard):
            w = assign[v]
            p = w * 128 + row_next[w]
            row_next[w] += 1
            pos_of_node[c, v] = p
            node_at_pos[c, p] = v

    bw = np.maximum((cnt + 127) // 128, 1).max(axis=0)  # blocks per window
    block_win = []
    for w in range(nwin):
        block_win += [w] * int(bw[w])
    B = len(block_win)

    # slot assignment: edges of (core, window) packed into that window's blocks
    eslot = np.full((NCORES, B, 128), -1, dtype=np.int64)
    wb0 = np.concatenate([[0], np.cumsum(bw)])  # first block of window w
    for c in range(NCORES):
        ec = np.where(owner == c)[0]
        wids = assigns[c][dst_local[ec]]
        for w in range(nwin):
            es = ec[wids == w]
            b0 = wb0[w]
            for j, e in enumerate(es):
                eslot[c, b0 + j // 128, j % 128] = e

    # host-side h0 (layer-0 table): h0 = x @ init_W + init_b
    h0 = x @ np.asarray(inputs["init_W"], np.float32) + np.asarray(
        inputs["init_b"], np.float32
    )

    # o-major reorder of edge MLP weights: col j = i*H + o -> o*H + i
    Wm = np.asarray(inputs["edge_mlp_W"], np.float32).reshape(L, EA, H, H)
    Wm = Wm.transpose(0, 1, 3, 2).reshape(L, EA, H * H)
    bm = np.asarray(inputs["edge_mlp_b"], np.float32).reshape(L, H, H)
    bm = bm.transpose(0, 2, 1).reshape(L, H * H)
    W_aug = np.concatenate([Wm, bm[:, None, :]], axis=1)  # [L, EA+1, H*H]

    rootW = np.asarray(inputs["root_W"], np.float32)
    root_aug = np.zeros((L, 2 * H + 1, H), np.float32)
    root_aug[:, :H, :] = rootW
    root_aug[:, 2 * H, :] = np.asarray(inputs["root_b"], np.float32)

    trans_aug = np.concatenate(
        [np.asarray(inputs["trans_W"], np.float32),
         np.asarray(inputs["trans_b"], np.float32)[:, None, :]], axis=1
    )  # [L, 2H+1, H]

    final_aug = np.zeros((2 * H + 1, T), np.float32)
    final_aug[:H, :] = np.asarray(inputs["final_W"], np.float32)
    final_aug[2 * H, :] = np.asarray(inputs["final_b"], np.float32)

    gn = np.concatenate(
        [np.asarray(inputs["gn_w"], np.float32),
         np.asarray(inputs["gn_b"], np.float32),
         np.asarray(inputs["gn_ms"], np.float32)], axis=1
    )[:, None, :]  # [L, 1, 3H]
    fgn = np.concatenate(
        [np.asarray(inputs["fgn_w"], np.float32),
         np.asarray(inputs["fgn_b"], np.float32),
         np.asarray(inputs["fgn_ms"], np.float32)], axis=0
    )[None, :]  # [1, 3T]

    iota = np.broadcast_to(np.arange(128, dtype=np.float32), (128, 128))
    ident = np.eye(128, dtype=np.float32)
    ones_row = np.ones((1, 128), np.float32)

    in_maps = []
    for c in range(NCORES):
        es = eslot[c]
        valid = es >= 0
        esc = np.where(valid, es, 0)
        flat = esc.reshape(-1)
        vflat = valid.reshape(-1)

        attrT_aug = np.zeros((EA + 1, B * 128), np.float32)
        attrT_aug[:EA, :] = edge_attr[flat].T * vflat
        attrT_aug[EA, :] = vflat.astype(np.float32)

        sg = src[flat]
        gidx = (sg // shard) * shard_pad + pos_of_node[sg // shard, sg % shard]
        gidx = np.where(vflat, gidx, 0).astype(np.int32)
        src_gidx = gidx.reshape(B, 128).T.copy()

        # pre-gathered layer-0 h[src] (pad slots zeroed)
        hsrc0 = (h0[sg] * vflat[:, None]).reshape(B, 128, H).transpose(1, 0, 2)

        wl = assigns[c][dst_local[flat]]
        dr = np.where(vflat,
                      pos_of_node[c, dst_local[flat]] - wl * 128, -1.0)
        dst_rel = dr.astype(np.float32).reshape(B, 128).T.copy()

        # own shard h0 in permuted layout [128, nwin, H]
        h0_own = np.zeros((shard_pad, H), np.float32)
        vmask = node_at_pos[c] >= 0
        h0_own[vmask] = h0[c * shard + node_at_pos[c, vmask]]
        h0_own = h0_own.reshape(nwin, 128, H).transpose(1, 0, 2)

        mask = (np.arange(128)[:, None] < wns[c][None, :]).astype(np.float32)

        in_maps.append(
            {
                "attrT_aug": np.ascontiguousarray(attrT_aug).astype(ml_dtypes.bfloat16),
                "src_gidx": np.ascontiguousarray(src_gidx),
                "dst_rel": np.ascontiguousarray(dst_rel),
                "hsrc0": np.ascontiguousarray(hsrc0).astype(ml_dtypes.bfloat16),
                "h0_own": np.ascontiguousarray(h0_own).astype(ml_dtypes.bfloat16),
                "mask": np.ascontiguousarray(mask),
                "W_aug": W_aug.astype(ml_dtypes.bfloat16),
                "root_aug": root_aug.astype(ml_dtypes.bfloat16),
                "trans_aug": trans_aug.astype(ml_dtypes.bfloat16),
                "final_aug": final_aug.astype(ml_dtypes.bfloat16),
                "gn": gn,
                "fgn": fgn,
                "iota": iota.astype(ml_dtypes.bfloat16),
                "ident": ident.astype(ml_dtypes.bfloat16),
                "ones_row": ones_row,
            }
        )

    shapes = dict(
        N=N, E=E, IN=IN, H=H, EA=EA, T=T, L=L, shard=shard,
        shard_pad=shard_pad, nub=nwin, B=B, block_win=tuple(block_win),
        bw=tuple(int(v) for v in bw), nwin=nwin,
    )
    perms = node_at_pos  # for output unpermute
    return in_maps, shapes, perms


# ------------------------------------------------------------- device build


def build_program(s):
    H, EA, T, L = s["H"], s["EA"], s["T"], s["L"]
    B, nub, nwin = s["B"], s["nub"], s["nwin"]
    shard_pad = s["shard_pad"]
    block_win = s["block_win"]
    bw = s["bw"]
    HH = H * H
    HHH = HH // 2
    n_total = shard_pad * NCORES
    n_real = s["N"]

    nc = bacc.Bacc("TRN2", target_bir_lowering=False, debug=False,
                   enable_asserts=False, num_devices=NCORES)

    def din(name, shape, dtype=mdt.float32):
        return nc.dram_tensor(name, shape, dtype, kind="ExternalInput").ap()

    attrT = din("attrT_aug", [EA + 1, B * 128], mdt.bfloat16)
    src_gidx = din("src_gidx", [128, B], mdt.int32)
    dst_rel = din("dst_rel", [128, B])
    hsrc0_in = din("hsrc0", [128, B, H], mdt.bfloat16)
    h0_own_in = din("h0_own", [128, nub, H], mdt.bfloat16)
    mask_in = din("mask", [128, nub])
    W_in = din("W_aug", [L, EA + 1, HH], mdt.bfloat16)
    root_in = din("root_aug", [L, 2 * H + 1, H], mdt.bfloat16)
    trans_in = din("trans_aug", [L, 2 * H + 1, H], mdt.bfloat16)
    final_in = din("final_aug", [2 * H + 1, T], mdt.bfloat16)
    gn_in = din("gn", [L, 1, 3 * H])
    fgn_in = din("fgn", [1, 3 * T])
    iota_in = din("iota", [128, 128], mdt.bfloat16)
    ident_in = din("ident", [128, 128], mdt.bfloat16)
    ones_in = din("ones_row", [1, 128])

    out_dram = nc.dram_tensor("out", [shard_pad, T], mdt.float32,
                              kind="ExternalOutput").ap()

    rg = [list(range(NCORES))]

    with tile.TileContext(nc) as tc:
        with (
            tc.tile_pool(name="const", bufs=1) as cpool,
            tc.tile_pool(name="hbuf", bufs=1) as hpool,
            tc.tile_pool(name="ew", bufs=RUNA) as ewpool,
            tc.tile_pool(name="tmp", bufs=4) as tmppool,
            tc.tile_pool(name="rows", bufs=10) as rpool,
            tc.tile_pool(name="ps", bufs=1, space="PSUM") as ps,
            tc.tile_pool(name="dram", bufs=1, space="DRAM") as dram,
        ):
            def load(pool, shape, ap, dtype=mdt.float32, tag=None):
                t = pool.tile(shape, dtype, tag=tag)
                nc.sync.dma_start(t[:], ap)
                return t

            attrT_sb = load(cpool, [EA + 1, B * 128], attrT[:], mdt.bfloat16,
                            tag="attrT")
            dst_sb = load(cpool, [128, B], dst_rel[:], tag="dstrel")
            idx_sb = load(cpool, [128, B], src_gidx[:], mdt.int32, tag="sidx")
            hsrc0_sb = load(cpool, [128, B, H], hsrc0_in[:], mdt.bfloat16,
                            tag="hsrc0")
            mask_sb = load(cpool, [128, nub], mask_in[:], tag="mask")
            final_sb = load(cpool, [2 * H + 1, T], final_in[:], mdt.bfloat16,
                            tag="finalw")
            iota_sb = load(cpool, [128, 128], iota_in[:], mdt.bfloat16,
                           tag="iota")
            ident_sb = load(cpool, [128, 128], ident_in[:], mdt.bfloat16,
                            tag="ident")
            onesr_sb = load(cpool, [1, 128], ones_in[:], tag="onesr")
            fgn_sb = load(cpool, [1, 3 * T], fgn_in[:], tag="fgn")
            W_l = [load(cpool, [EA + 1, HH], W_in[li], mdt.bfloat16,
                        tag=f"W{li}") for li in range(L)]
            root_l = [load(cpool, [2 * H + 1, H], root_in[li], mdt.bfloat16,
                           tag=f"rw{li}") for li in range(L)]
            trans_l = [load(cpool, [2 * H + 1, H], trans_in[li], mdt.bfloat16,
                            tag=f"tw{li}") for li in range(L)]
            gn_l = [load(cpool, [1, 3 * H], gn_in[li], tag=f"gn{li}")
                    for li in range(L)]

            # ---- persistent tiles
            hA = hpool.tile([128, nub, H], mdt.bfloat16)
            hB = hpool.tile([128, nub, H], mdt.bfloat16)
            hc_all = hpool.tile([128, nub, H], mdt.bfloat16)
            t1_all = hpool.tile([128, nub, H], mdt.float32)
            conv_sb = hpool.tile([128, nub, H], mdt.float32)
            st_sb = hpool.tile([128, nub, 2 * H], mdt.float32)
            fst_sb = hpool.tile([128, nub, 2 * T], mdt.float32)
            hsrc_sb = hpool.tile([128, B, H], mdt.bfloat16)
            catT_all = hpool.tile([2 * H + 1, nub, 128], mdt.bfloat16)
            fo_sb = hpool.tile([128, nub, T], mdt.float32)
            y_sb = hpool.tile([128, nub, T], mdt.float32)
            cd_sb = hpool.tile([128, 2 * H], mdt.float32)
            fcd_sb = hpool.tile([128, 2 * T], mdt.float32)
            stats_sb = hpool.tile([1, 2 * H], mdt.float32)
            fstats_sb = hpool.tile([1, 2 * T], mdt.float32)

            # one-hot blocks (layer-invariant, bf16; built lazily on Pool
            # inside the layer-0 edge loop to avoid a startup queue jam)
            onehot_sb = cpool.tile([128, B, 128], mdt.bfloat16)

            nc.vector.memset(catT_all[H : 2 * H, :, :], 0.0)
            nc.vector.memset(catT_all[2 * H : 2 * H + 1, :, :], 1.0)
            nc.sync.dma_start(hA[:], h0_own_in[:])

            hstage_dram = dram.tile([shard_pad, H], mdt.bfloat16)
            htable_l = [None] + [
                dram.tile([n_total, H], mdt.bfloat16, addr_space="Shared",
                          tag=f"htable{li}", name=f"htable{li}")
                for li in range(1, L)
            ]
            st_in = dram.tile([1, 2 * H], mdt.float32)
            st_out_l = [dram.tile([1, 2 * H], mdt.float32, addr_space="Shared",
                                  tag=f"stout{li}", name=f"stout{li}")
                        for li in range(L)]
            fst_in = dram.tile([1, 2 * T], mdt.float32)
            fst_out = dram.tile([1, 2 * T], mdt.float32, addr_space="Shared")

            hstage_v = hstage_dram[:].rearrange("(u p) f -> p u f", p=128)

            def rstd_row(dstrow, varrow, width, tag):
                """dstrow = 1/sqrt(varrow+EPS) via reciprocal+sqrt+Newton."""
                ve = rpool.tile([1, width], mdt.float32, tag=tag)
                nc.vector.tensor_scalar_add(ve[:], varrow, EPS)
                r2 = rpool.tile([1, width], mdt.float32, tag=tag)
                nc.vector.reciprocal(r2[:], ve[:])
                r0 = rpool.tile([1, width], mdt.float32, tag=tag)
                nc.scalar.activation(r0[:], r2[:], AF.Sqrt)
                t0 = rpool.tile([1, width], mdt.float32, tag=tag)
                nc.vector.tensor_mul(t0[:], r0[:], r0[:])
                nc.vector.tensor_mul(t0[:], t0[:], ve[:])
                nc.vector.scalar_tensor_tensor(
                    t0[:], t0[:], -0.5, r0[:], op0=ALU.mult, op1=ALU.mult
                )
                nc.vector.scalar_tensor_tensor(
                    dstrow, r0[:], 1.5, t0[:], op0=ALU.mult, op1=ALU.add
                )

            def cd_rows(crow, srow, gnw, gnb, gnms, width, tag):
                """crow[0:w] = C = rstd*w ; crow[w:2w] = D = b - ms*mean*C."""
                mean = rpool.tile([1, width], mdt.float32, tag=tag)
                nc.vector.tensor_scalar_mul(mean[:], srow[:, width : 2 * width],
                                            1.0 / n_real)
                msq = rpool.tile([1, width], mdt.float32, tag=tag)
                nc.vector.tensor_scalar_mul(msq[:], srow[:, 0:width],
                                            1.0 / n_real)
                mm = rpool.tile([1, width], mdt.float32, tag=tag)
                nc.vector.tensor_mul(mm[:], mean[:], mean[:])
                nc.vector.tensor_mul(mm[:], mm[:], gnms)
                co = rpool.tile([1, width], mdt.float32, tag=tag)
                nc.vector.tensor_scalar(co[:], gnms, -1.0, 2.0, op0=ALU.mult,
                                        op1=ALU.add)
                nc.vector.tensor_mul(mm[:], mm[:], co[:])
                var = rpool.tile([1, width], mdt.float32, tag=tag)
                nc.vector.tensor_sub(var[:], msq[:], mm[:])
                rstd = rpool.tile([1, width], mdt.float32, tag=tag)
                rstd_row(rstd[:], var[:], width, tag)
                nc.vector.tensor_mul(crow[:, 0:width], rstd[:], gnw)
                nc.vector.tensor_mul(crow[:, width : 2 * width], mean[:], gnms)
                nc.vector.tensor_mul(crow[:, width : 2 * width],
                                     crow[:, width : 2 * width],
                                     crow[:, 0:width])
                nc.vector.scalar_tensor_tensor(
                    crow[:, width : 2 * width], crow[:, width : 2 * width],
                    -1.0, gnb, op0=ALU.mult, op1=ALU.add,
                )

            def emit_mlp_relu(li, b):
                """MLP matmuls + Act relu -> bf16 SBUF ring tile."""
                pre = ps.tile([128, HH], mdt.float32, tag="pre", bufs=2)
                a_sl = attrT_sb[:, b * 128 : (b + 1) * 128]
                nc.tensor.matmul(pre[:, 0:HHH], a_sl, W_l[li][:, 0:HHH],
                                 start=True, stop=True)
                nc.tensor.matmul(pre[:, HHH:HH], a_sl, W_l[li][:, HHH:HH],
                                 start=True, stop=True)
                ew = ewpool.tile([128, HH], mdt.bfloat16, tag="ew")
                nc.scalar.activation(ew[:], pre[:], AF.Relu)
                return ew

            def emit_mlp_raw(li, b):
                pre = ps.tile([128, HH], mdt.float32, tag="pre", bufs=2)
                a_sl = attrT_sb[:, b * 128 : (b + 1) * 128]
                nc.tensor.matmul(pre[:, 0:HHH], a_sl, W_l[li][:, 0:HHH],
                                 start=True, stop=True)
                nc.tensor.matmul(pre[:, HHH:HH], a_sl, W_l[li][:, HHH:HH],
                                 start=True, stop=True)
                return pre

            def path_for(li, b):
                # A: Act relu + DVE mult; P: Act relu + Pool mult; D: DVE fused
                if li == 0:
                    return ("A", "P", "D", "P")[b % 4]
                return "A"

            pending = {}
            hcur, hnxt = hA, hB

            for li in range(L):
                hsrc_v = hsrc0_sb if li == 0 else hsrc_sb
                if li > 0:
                    htable = htable_l[li]
                    for b in range(B):
                        nc.gpsimd.indirect_dma_start(
                            out=hsrc_sb[:, b, :],
                            out_offset=None,
                            in_=htable[:],
                            in_offset=bass.IndirectOffsetOnAxis(
                                ap=idx_sb[:, b : b + 1], axis=0
                            ),
                        )

                # ---- edge phase with interleaved node pass 1
                nc.vector.memset(stats_sb[:], 0.0)
                b = 0
                for w in range(nwin):
                    aggI = ps.tile([128, HH], mdt.float32, tag="agg", bufs=1)
                    for j in range(bw[w]):
                        path = path_for(li, b)
                        first, last = j == 0, j == bw[w] - 1
                        h_bc = (hsrc_v[:, b, :].unsqueeze(1)
                                .broadcast_to([128, H, H]))
                        tmp = tmppool.tile([128, HH], mdt.bfloat16, tag="tmp")
                        tmp_v = tmp[:].rearrange("p (o i) -> p o i", o=H, i=H)
                        if path == "D":
                            pre = emit_mlp_raw(li, b)
                            nc.vector.scalar_tensor_tensor(
                                tmp_v,
                                pre[:].rearrange("p (o i) -> p o i", o=H, i=H),
                                0.0, h_bc, op0=ALU.max, op1=ALU.mult,
                            )
                        else:
                            if (li, b) in pending:
                                ew = pending.pop((li, b))
                            else:
                                ew = emit_mlp_relu(li, b)
                            ew_v = ew[:].rearrange("p (o i) -> p o i", o=H,
                                                   i=H)
                            if path == "P":
                                nc.gpsimd.tensor_tensor(tmp_v, ew_v, h_bc,
                                                        op=ALU.mult)
                            else:
                                nc.vector.tensor_tensor(tmp_v, ew_v, h_bc,
                                                        op=ALU.mult)
                        oh = onehot_sb[:, b, :]
                        if li == 0:
                            nc.gpsimd.tensor_scalar(
                                oh, iota_sb[:], dst_sb[:, b : b + 1],
                                None, op0=ALU.is_equal,
                            )
                        nc.tensor.matmul(aggI[:, 0:HHH], oh, tmp[:, 0:HHH],
                                         start=first, stop=last)
                        nc.tensor.matmul(aggI[:, HHH:HH], oh, tmp[:, HHH:HH],
                                         start=first, stop=last)
                        b += 1

                    # ---- window wrapup (node pass 1)
                    tp = ps.tile([H, 128], mdt.bfloat16, tag="node", bufs=2)
                    nc.tensor.transpose(tp[:], hcur[:, w, :], ident_sb[:])
                    nc.scalar.activation(catT_all[0:H, w, :], tp[:], AF.Copy)
                    rt = ps.tile([128, H], mdt.float32, tag="node", bufs=2)
                    nc.tensor.matmul(rt[:], catT_all[:, w, :], root_l[li][:],
                                     start=True, stop=True)
                    nc.vector.tensor_reduce(
                        conv_sb[:, w, :],
                        aggI[:].rearrange("p (o i) -> p o i", o=H, i=H),
                        axis=mybir.AxisListType.X, op=ALU.add,
                    )
                    nc.vector.tensor_add(conv_sb[:, w, :], conv_sb[:, w, :],
                                         rt[:])
                    nc.vector.tensor_mul(st_sb[:, w, 0:H], conv_sb[:, w, :],
                                         conv_sb[:, w, :])
                    nc.vector.tensor_copy(st_sb[:, w, H : 2 * H],
                                          conv_sb[:, w, :])
                    smm = ps.tile([1, 2 * H], mdt.float32, tag="node", bufs=2)
                    nc.tensor.matmul(smm[:], mask_sb[:, w : w + 1],
                                     st_sb[:, w, :], start=True, stop=True)
                    nc.vector.tensor_add(stats_sb[:], stats_sb[:], smm[:])

                # ---- stats AllReduce
                nc.sync.dma_start(st_in[:], stats_sb[:])
                st_out = st_out_l[li]
                nc.gpsimd.collective_compute(
                    "AllReduce", ALU.add, replica_groups=rg,
                    ins=[st_in.opt()], outs=[st_out.opt()],
                )

                # ---- next-layer MLP runahead (fills the AllReduce window)
                if li + 1 < L:
                    for rb in range(RUNA):
                        pending[(li + 1, rb)] = emit_mlp_relu(li + 1, rb)

                srow2 = rpool.tile([1, 2 * H], mdt.float32, tag="srow")
                nc.sync.dma_start(srow2[:], st_out[:])

                # ---- C/D rows + broadcast
                crow = rpool.tile([1, 2 * H], mdt.float32, tag="cdrow")
                cd_rows(crow, srow2, gn_l[li][:, 0:H], gn_l[li][:, H : 2 * H],
                        gn_l[li][:, 2 * H : 3 * H], H, "nrow")
                cd_ps = ps.tile([128, 2 * H], mdt.float32, tag="node", bufs=2)
                nc.tensor.matmul(cd_ps[:], onesr_sb[:], crow[:], start=True,
                                 stop=True)
                nc.scalar.activation(cd_sb[:], cd_ps[:], AF.Copy)

                # ---- node pass 2 (batched)
                nc.vector.tensor_tensor(
                    t1_all[:], conv_sb[:],
                    cd_sb[:, 0:H].unsqueeze(1).broadcast_to([128, nub, H]),
                    op=ALU.mult)
                nc.vector.tensor_tensor(
                    t1_all[:], t1_all[:],
                    cd_sb[:, H : 2 * H].unsqueeze(1)
                    .broadcast_to([128, nub, H]),
                    op=ALU.add)
                nc.vector.scalar_tensor_tensor(
                    hc_all[:], t1_all[:], 0.0, hcur[:],
                    op0=ALU.max, op1=ALU.add)
                for u in range(nub):
                    tp2 = ps.tile([H, 128], mdt.bfloat16, tag="node", bufs=2)
                    nc.tensor.transpose(tp2[:], hc_all[:, u, :], ident_sb[:])
                    nc.scalar.activation(catT_all[H : 2 * H, u, :], tp2[:],
                                         AF.Copy)
                    tr = ps.tile([128, H], mdt.float32, tag="node", bufs=2)
                    nc.tensor.matmul(tr[:], catT_all[:, u, :], trans_l[li][:],
                                     start=True, stop=True)
                    nc.scalar.activation(hnxt[:, u, :], tr[:], AF.Relu)

                if li + 1 < L:
                    nc.sync.dma_start(hstage_v, hnxt[:])
                    nc.gpsimd.collective_compute(
                        "AllGather", ALU.bypass, replica_groups=rg,
                        ins=[hstage_dram.opt()], outs=[htable_l[li + 1].opt()],
                    )

                hcur, hnxt = hnxt, hcur

            # ============ final (fused) ============
            nc.vector.memset(fstats_sb[:], 0.0)
            for u in range(nub):
                tp3 = ps.tile([H, 128], mdt.bfloat16, tag="node", bufs=2)
                nc.tensor.transpose(tp3[:], hcur[:, u, :], ident_sb[:])
                nc.scalar.activation(catT_all[0:H, u, :], tp3[:], AF.Copy)
                f_ps = ps.tile([128, T], mdt.float32, tag="node", bufs=2)
                nc.tensor.matmul(f_ps[:], catT_all[:, u, :], final_sb[:],
                                 start=True, stop=True)
                nc.scalar.activation(fo_sb[:, u, :], f_ps[:], AF.Copy)
                nc.vector.tensor_mul(fst_sb[:, u, 0:T], fo_sb[:, u, :],
                                     fo_sb[:, u, :])
                nc.vector.tensor_copy(fst_sb[:, u, T : 2 * T], fo_sb[:, u, :])
                fsmm = ps.tile([1, 2 * T], mdt.float32, tag="node", bufs=2)
                nc.tensor.matmul(fsmm[:], mask_sb[:, u : u + 1],
                                 fst_sb[:, u, :], start=True, stop=True)
                nc.vector.tensor_add(fstats_sb[:], fstats_sb[:], fsmm[:])

            nc.sync.dma_start(fst_in[:], fstats_sb[:])
            nc.gpsimd.collective_compute(
                "AllReduce", ALU.add, replica_groups=rg,
                ins=[fst_in.opt()], outs=[fst_out.opt()],
            )
            fsrow2 = rpool.tile([1, 2 * T], mdt.float32, tag="fsrow")
            nc.sync.dma_start(fsrow2[:], fst_out[:])

            fcrow = rpool.tile([1, 2 * T], mdt.float32, tag="fcdrow")
            cd_rows(fcrow, fsrow2, fgn_sb[:, 0:T], fgn_sb[:, T : 2 * T],
                    fgn_sb[:, 2 * T : 3 * T], T, "frow")
            fcd_ps = ps.tile([128, 2 * T], mdt.float32, tag="node", bufs=2)
            nc.tensor.matmul(fcd_ps[:], onesr_sb[:], fcrow[:], start=True,
                             stop=True)
            nc.scalar.activation(fcd_sb[:], fcd_ps[:], AF.Copy)

            nc.vector.tensor_tensor(
                y_sb[:], fo_sb[:],
                fcd_sb[:, 0:T].unsqueeze(1).broadcast_to([128, nub, T]),
                op=ALU.mult)
            nc.vector.tensor_tensor(
                y_sb[:], y_sb[:],
                fcd_sb[:, T : 2 * T].unsqueeze(1).broadcast_to([128, nub, T]),
                op=ALU.add)
            nc.vector.tensor_scalar_max(y_sb[:], y_sb[:], 0.0)
            out_v = out_dram.rearrange("(u p) f -> p u f", p=128)
            nc.sync.dma_start(out_v, y_sb[:])

    nc.compile()
    return nc


# ------------------------------------------------------------------ driver

_CACHE = {}


def kernel(**inputs) -> np.ndarray:
    in_maps, s, node_at_pos = prep_inputs(inputs)
    key = (s["N"], s["E"], s["B"], s["block_win"])
    if key not in _CACHE:
        _CACHE[key] = build_program(s)
    nc = _CACHE[key]

    from concourse.bass_utils import run_bass_kernel_spmd

    res = run_bass_kernel_spmd(nc, in_maps, core_ids=list(range(NCORES)))
    shard, T, N = s["shard"], s["T"], s["N"]
    out = np.empty((N, T), np.float32)
    for c in range(NCORES):
        rows = res.results[c]["out"]
        vmask = node_at_pos[c] >= 0
        out[c * shard + node_at_pos[c, vmask]] = rows[vmask]
    return out.astype(np.float32)


# revision 20
# speedup vs baseline: 1.4579x; 1.4579x over previous
"""DJMGNN (NNConv/GraphNorm GNN) Trainium2 kernel, 8-core SPMD. v2.

Sharding: nodes range-sharded N/8 per core, then PERMUTED within each shard so
every 128-node dst window holds <=512 edges (uniform 4 blocks/window, B=80
vs 98 unbalanced). Edges assigned to the core owning their dst node.

Per layer:
  - h table in DRAM; per-block indirect-DMA gather of h[src] (Pool engine).
    Layer 0 needs no table/gather: h0 = x@W is computed on HOST and shipped
    pre-gathered (hsrc0) plus the local shard (h0_own).
  - edge MLP on PE (attrT stationary, 2x512-col matmuls into one 2-bank PSUM
    tile) -> Act relu -> bf16 SBUF ring.
  - relu*h mult on DVE (bf16 2x) or Pool (layer 0), o-major broadcast AP.
  - one-hot scatter-matmul accumulating aggI[u,(o,i)] per 128-dst window.
  - interleaved node pass 1 at each window close: DVE strided i-reduce,
    root matmul (catT stationary), conv, stats via masked-ones matmul.
  - stats AllReduce; next-layer MLP runahead emitted before it to hide
    collective latency; batched pass 2; AllGather of the new h shard.
  - final transition + GraphNorm fused into layer-2 pass 2.
"""

import sys

if "/opt/trn_rl_repo" not in sys.path:
    sys.path.insert(0, "/opt/trn_rl_repo")

import numpy as np
import ml_dtypes

import concourse.bass as bass
import concourse.bacc as bacc
import concourse.mybir as mybir
import concourse.tile as tile

mdt = mybir.dt
AF = mybir.ActivationFunctionType
ALU = mybir.AluOpType

NCORES = 8
EPS = 1e-5
RUNA = 8  # next-layer MLP runahead blocks (hide stats-AllReduce latency)


# ---------------------------------------------------------------- host prep


def _balance_windows(deg, nwin, cap_n=128):
    """Assign local nodes to nwin windows, balancing edge load (greedy LPT)."""
    order = np.argsort(-deg, kind="stable")
    wload = np.zeros(nwin, dtype=np.int64)
    wn = np.zeros(nwin, dtype=np.int64)
    assign = np.full(deg.shape[0], -1, dtype=np.int64)
    for v in order:
        cand = np.where(wn < cap_n)[0]
        w = cand[np.argmin(wload[cand])]
        assign[v] = w
        wload[w] += deg[v]
        wn[w] += 1
    return assign, wload, wn


def prep_inputs(inputs):
    x = np.asarray(inputs["x"], np.float32)
    edge_attr = np.asarray(inputs["edge_attr"], np.float32)
    edge_index = np.asarray(inputs["edge_index"])
    N, IN = x.shape
    E, EA = edge_attr.shape
    H = np.asarray(inputs["init_W"]).shape[1]
    L = np.asarray(inputs["edge_mlp_W"]).shape[0]
    T = np.asarray(inputs["final_W"]).shape[1]
    shard = N // NCORES
    nwin = (shard + 127) // 128
    shard_pad = nwin * 128
    src = edge_index[0].astype(np.int64)
    dst = edge_index[1].astype(np.int64)
    owner = dst // shard
    dst_local = dst - owner * shard

    # per-core window assignment (node permutation)
    assigns, wns = [], []
    pos_of_node = np.empty((NCORES, shard), np.int64)  # node -> padded position
    node_at_pos = np.full((NCORES, shard_pad), -1, np.int64)
    cnt = np.zeros((NCORES, nwin), np.int64)
    for c in range(NCORES):
        dl = dst_local[owner == c]
        deg = np.bincount(dl, minlength=shard)
        assign, wload, wn = _balance_windows(deg, nwin)
        assigns.append(assign)
        wns.append(wn)
        cnt[c] = wload
        row_next = np.zeros(nwin, np.int64)
        for v in range(shard):
            w = assign[v]
            p = w * 128 + row_next[w]
            row_next[w] += 1
            pos_of_node[c, v] = p
            node_at_pos[c, p] = v

    bw = np.maximum((cnt + 127) // 128, 1).max(axis=0)  # blocks per window
    block_win = []
    for w in range(nwin):
        block_win += [w] * int(bw[w])
    B = len(block_win)

    # slot assignment: edges of (core, window) packed into that window's blocks
    eslot = np.full((NCORES, B, 128), -1, dtype=np.int64)
    wb0 = np.concatenate([[0], np.cumsum(bw)])  # first block of window w
    for c in range(NCORES):
        ec = np.where(owner == c)[0]
        wids = assigns[c][dst_local[ec]]
        for w in range(nwin):
            es = ec[wids == w]
            b0 = wb0[w]
            for j, e in enumerate(es):
                eslot[c, b0 + j // 128, j % 128] = e

    # host-side h0 (layer-0 table): h0 = x @ init_W + init_b
    h0 = x @ np.asarray(inputs["init_W"], np.float32) + np.asarray(
        inputs["init_b"], np.float32
    )

    # o-major reorder of edge MLP weights: col j = i*H + o -> o*H + i
    Wm = np.asarray(inputs["edge_mlp_W"], np.float32).reshape(L, EA, H, H)
    Wm = Wm.transpose(0, 1, 3, 2).reshape(L, EA, H * H)
    bm = np.asarray(inputs["edge_mlp_b"], np.float32).reshape(L, H, H)
    bm = bm.transpose(0, 2, 1).reshape(L, H * H)
    W_aug = np.concatenate([Wm, bm[:, None, :]], axis=1)  # [L, EA+1, H*H]

    rootW = np.asarray(inputs["root_W"], np.float32)
    root_aug = np.zeros((L, 2 * H + 1, H), np.float32)
    root_aug[:, :H, :] = rootW
    root_aug[:, 2 * H, :] = np.asarray(inputs["root_b"], np.float32)

    trans_aug = np.concatenate(
        [np.asarray(inputs["trans_W"], np.float32),
         np.asarray(inputs["trans_b"], np.float32)[:, None, :]], axis=1
    )  # [L, 2H+1, H]

    final_aug = np.zeros((2 * H + 1, T), np.float32)
    final_aug[:H, :] = np.asarray(inputs["final_W"], np.float32)
    final_aug[2 * H, :] = np.asarray(inputs["final_b"], np.float32)

    gn = np.concatenate(
        [np.asarray(inputs["gn_w"], np.float32),
         np.asarray(inputs["gn_b"], np.float32),
         np.asarray(inputs["gn_ms"], np.float32)], axis=1
    )[:, None, :]  # [L, 1, 3H]
    fgn = np.concatenate(
        [np.asarray(inputs["fgn_w"], np.float32),
         np.asarray(inputs["fgn_b"], np.float32),
         np.asarray(inputs["fgn_ms"], np.float32)], axis=0
    )[None, :]  # [1, 3T]

    iota = np.broadcast_to(np.arange(128, dtype=np.float32), (128, 128))
    ident = np.eye(128, dtype=np.float32)
    ones_row = np.ones((1, 128), np.float32)

    in_maps = []
    for c in range(NCORES):
        es = eslot[c]
        valid = es >= 0
        esc = np.where(valid, es, 0)
        flat = esc.reshape(-1)
        vflat = valid.reshape(-1)

        attrT_aug = np.zeros((EA + 1, B * 128), np.float32)
        attrT_aug[:EA, :] = edge_attr[flat].T * vflat
        attrT_aug[EA, :] = vflat.astype(np.float32)

        sg = src[flat]
        gidx = (sg // shard) * shard_pad + pos_of_node[sg // shard, sg % shard]
        gidx = np.where(vflat, gidx, 0).astype(np.int32)
        src_gidx = gidx.reshape(B, 128).T.copy()

        # pre-gathered layer-0 h[src] (pad slots zeroed)
        hsrc0 = (h0[sg] * vflat[:, None]).reshape(B, 128, H).transpose(1, 0, 2)

        wl = assigns[c][dst_local[flat]]
        dr = np.where(vflat,
                      pos_of_node[c, dst_local[flat]] - wl * 128, -1.0)
        dst_rel = dr.astype(np.float32).reshape(B, 128).T
        # host-built one-hot scatter blocks [128 slot, B, 128 dstrow]
        onehot = (np.arange(128, dtype=np.float32)[None, None, :]
                  == dst_rel[:, :, None])

        # own shard h0 in permuted layout [128, nwin, H]
        h0_own = np.zeros((shard_pad, H), np.float32)
        vmask = node_at_pos[c] >= 0
        h0_own[vmask] = h0[c * shard + node_at_pos[c, vmask]]
        h0_own = h0_own.reshape(nwin, 128, H).transpose(1, 0, 2)

        mask = (np.arange(128)[:, None] < wns[c][None, :]).astype(np.float32)

        in_maps.append(
            {
                "attrT_aug": np.ascontiguousarray(attrT_aug).astype(ml_dtypes.bfloat16),
                "src_gidx": np.ascontiguousarray(src_gidx),
                "onehot": np.ascontiguousarray(onehot).astype(ml_dtypes.bfloat16),
                "hsrc0": np.ascontiguousarray(hsrc0).astype(ml_dtypes.bfloat16),
                "h0_own": np.ascontiguousarray(h0_own).astype(ml_dtypes.bfloat16),
                "mask": np.ascontiguousarray(mask),
                "W_aug": W_aug.astype(ml_dtypes.bfloat16),
                "root_aug": root_aug.astype(ml_dtypes.bfloat16),
                "trans_aug": trans_aug.astype(ml_dtypes.bfloat16),
                "final_aug": final_aug.astype(ml_dtypes.bfloat16),
                "gn": gn,
                "fgn": fgn,
                "ident": ident.astype(ml_dtypes.bfloat16),
                "ones_row": ones_row,
            }
        )

    shapes = dict(
        N=N, E=E, IN=IN, H=H, EA=EA, T=T, L=L, shard=shard,
        shard_pad=shard_pad, nub=nwin, B=B, block_win=tuple(block_win),
        bw=tuple(int(v) for v in bw), nwin=nwin,
    )
    perms = node_at_pos  # for output unpermute
    return in_maps, shapes, perms


# ------------------------------------------------------------- device build


def build_program(s):
    H, EA, T, L = s["H"], s["EA"], s["T"], s["L"]
    B, nub, nwin = s["B"], s["nub"], s["nwin"]
    shard_pad = s["shard_pad"]
    block_win = s["block_win"]
    bw = s["bw"]
    HH = H * H
    HHH = HH // 2
    n_total = shard_pad * NCORES
    n_real = s["N"]

    nc = bacc.Bacc("TRN2", target_bir_lowering=False, debug=False,
                   enable_asserts=False, num_devices=NCORES)

    def din(name, shape, dtype=mdt.float32):
        return nc.dram_tensor(name, shape, dtype, kind="ExternalInput").ap()

    attrT = din("attrT_aug", [EA + 1, B * 128], mdt.bfloat16)
    src_gidx = din("src_gidx", [128, B], mdt.int32)
    onehot_in = din("onehot", [128, B, 128], mdt.bfloat16)
    hsrc0_in = din("hsrc0", [128, B, H], mdt.bfloat16)
    h0_own_in = din("h0_own", [128, nub, H], mdt.bfloat16)
    mask_in = din("mask", [128, nub])
    W_in = din("W_aug", [L, EA + 1, HH], mdt.bfloat16)
    root_in = din("root_aug", [L, 2 * H + 1, H], mdt.bfloat16)
    trans_in = din("trans_aug", [L, 2 * H + 1, H], mdt.bfloat16)
    final_in = din("final_aug", [2 * H + 1, T], mdt.bfloat16)
    gn_in = din("gn", [L, 1, 3 * H])
    fgn_in = din("fgn", [1, 3 * T])
    ident_in = din("ident", [128, 128], mdt.bfloat16)
    ones_in = din("ones_row", [1, 128])

    out_dram = nc.dram_tensor("out", [shard_pad, T], mdt.float32,
                              kind="ExternalOutput").ap()

    rg = [list(range(NCORES))]

    with tile.TileContext(nc) as tc:
        with (
            tc.tile_pool(name="const", bufs=1) as cpool,
            tc.tile_pool(name="hbuf", bufs=1) as hpool,
            tc.tile_pool(name="ew", bufs=RUNA) as ewpool,
            tc.tile_pool(name="tmp", bufs=4) as tmppool,
            tc.tile_pool(name="rows", bufs=10) as rpool,
            tc.tile_pool(name="ps", bufs=1, space="PSUM") as ps,
            tc.tile_pool(name="dram", bufs=1, space="DRAM") as dram,
        ):
            def load(pool, shape, ap, dtype=mdt.float32, tag=None):
                t = pool.tile(shape, dtype, tag=tag)
                nc.sync.dma_start(t[:], ap)
                return t

            attrT_sb = load(cpool, [EA + 1, B * 128], attrT[:], mdt.bfloat16,
                            tag="attrT")
            idx_sb = load(cpool, [128, B], src_gidx[:], mdt.int32, tag="sidx")
            onehot_sb = load(cpool, [128, B, 128], onehot_in[:], mdt.bfloat16,
                             tag="onehot")
            hsrc0_sb = load(cpool, [128, B, H], hsrc0_in[:], mdt.bfloat16,
                            tag="hsrc0")
            mask_sb = load(cpool, [128, nub], mask_in[:], tag="mask")
            final_sb = load(cpool, [2 * H + 1, T], final_in[:], mdt.bfloat16,
                            tag="finalw")
            ident_sb = load(cpool, [128, 128], ident_in[:], mdt.bfloat16,
                            tag="ident")
            onesr_sb = load(cpool, [1, 128], ones_in[:], tag="onesr")
            fgn_sb = load(cpool, [1, 3 * T], fgn_in[:], tag="fgn")
            W_l = [load(cpool, [EA + 1, HH], W_in[li], mdt.bfloat16,
                        tag=f"W{li}") for li in range(L)]
            root_l = [load(cpool, [2 * H + 1, H], root_in[li], mdt.bfloat16,
                           tag=f"rw{li}") for li in range(L)]
            trans_l = [load(cpool, [2 * H + 1, H], trans_in[li], mdt.bfloat16,
                            tag=f"tw{li}") for li in range(L)]
            gn_l = [load(cpool, [1, 3 * H], gn_in[li], tag=f"gn{li}")
                    for li in range(L)]

            # ---- persistent tiles
            hA = hpool.tile([128, nub, H], mdt.bfloat16)
            hB = hpool.tile([128, nub, H], mdt.bfloat16)
            hc_all = hpool.tile([128, nub, H], mdt.bfloat16)
            t1_all = hpool.tile([128, nub, H], mdt.float32)
            conv_sb = hpool.tile([128, nub, H], mdt.float32)
            st_sb = hpool.tile([128, nub, 2 * H], mdt.float32)
            fst_sb = hpool.tile([128, nub, 2 * T], mdt.float32)
            hsrc_sb = hpool.tile([128, B, H], mdt.bfloat16)
            catT_all = hpool.tile([2 * H + 1, nub, 128], mdt.bfloat16)
            fo_sb = hpool.tile([128, nub, T], mdt.float32)
            y_sb = hpool.tile([128, nub, T], mdt.float32)
            cd_sb = hpool.tile([128, 2 * H], mdt.float32)
            fcd_sb = hpool.tile([128, 2 * T], mdt.float32)
            stats_sb = hpool.tile([1, 2 * H], mdt.float32)
            fstats_sb = hpool.tile([1, 2 * T], mdt.float32)

            nc.vector.memset(catT_all[H : 2 * H, :, :], 0.0)
            nc.vector.memset(catT_all[2 * H : 2 * H + 1, :, :], 1.0)
            nc.sync.dma_start(hA[:], h0_own_in[:])

            # warmup collective: absorbs first-collective latency during
            # the layer-0 edge phase (result unused)
            warm_in = dram.tile([1, 8], mdt.float32)
            warm_out = dram.tile([1, 8], mdt.float32, addr_space="Shared")
            wrow = rpool.tile([1, 8], mdt.float32, tag="warm")
            nc.vector.memset(wrow[:], 0.0)
            nc.sync.dma_start(warm_in[:], wrow[:])
            nc.gpsimd.collective_compute(
                "AllReduce", ALU.add, replica_groups=rg,
                ins=[warm_in.opt()], outs=[warm_out.opt()],
            )

            hstage_dram = dram.tile([shard_pad, H], mdt.bfloat16)
            htable_l = [None] + [
                dram.tile([n_total, H], mdt.bfloat16, addr_space="Shared",
                          tag=f"htable{li}", name=f"htable{li}")
                for li in range(1, L)
            ]
            st_in = dram.tile([1, 2 * H], mdt.float32)
            st_out_l = [dram.tile([1, 2 * H], mdt.float32, addr_space="Shared",
                                  tag=f"stout{li}", name=f"stout{li}")
                        for li in range(L)]
            fst_in = dram.tile([1, 2 * T], mdt.float32)
            fst_out = dram.tile([1, 2 * T], mdt.float32, addr_space="Shared")

            hstage_v = hstage_dram[:].rearrange("(u p) f -> p u f", p=128)

            def rstd_row(dstrow, varrow, width, tag):
                """dstrow = 1/sqrt(varrow+EPS) via reciprocal+sqrt+Newton."""
                ve = rpool.tile([1, width], mdt.float32, tag=tag)
                nc.vector.tensor_scalar_add(ve[:], varrow, EPS)
                r2 = rpool.tile([1, width], mdt.float32, tag=tag)
                nc.vector.reciprocal(r2[:], ve[:])
                r0 = rpool.tile([1, width], mdt.float32, tag=tag)
                nc.scalar.activation(r0[:], r2[:], AF.Sqrt)
                t0 = rpool.tile([1, width], mdt.float32, tag=tag)
                nc.vector.tensor_mul(t0[:], r0[:], r0[:])
                nc.vector.tensor_mul(t0[:], t0[:], ve[:])
                nc.vector.scalar_tensor_tensor(
                    t0[:], t0[:], -0.5, r0[:], op0=ALU.mult, op1=ALU.mult
                )
                nc.vector.scalar_tensor_tensor(
                    dstrow, r0[:], 1.5, t0[:], op0=ALU.mult, op1=ALU.add
                )

            def cd_rows(crow, srow, gnw, gnb, gnms, width, tag):
                """crow[0:w] = C = rstd*w ; crow[w:2w] = D = b - ms*mean*C."""
                mean = rpool.tile([1, width], mdt.float32, tag=tag)
                nc.vector.tensor_scalar_mul(mean[:], srow[:, width : 2 * width],
                                            1.0 / n_real)
                msq = rpool.tile([1, width], mdt.float32, tag=tag)
                nc.vector.tensor_scalar_mul(msq[:], srow[:, 0:width],
                                            1.0 / n_real)
                mm = rpool.tile([1, width], mdt.float32, tag=tag)
                nc.vector.tensor_mul(mm[:], mean[:], mean[:])
                nc.vector.tensor_mul(mm[:], mm[:], gnms)
                co = rpool.tile([1, width], mdt.float32, tag=tag)
                nc.vector.tensor_scalar(co[:], gnms, -1.0, 2.0, op0=ALU.mult,
                                        op1=ALU.add)
                nc.vector.tensor_mul(mm[:], mm[:], co[:])
                var = rpool.tile([1, width], mdt.float32, tag=tag)
                nc.vector.tensor_sub(var[:], msq[:], mm[:])
                rstd = rpool.tile([1, width], mdt.float32, tag=tag)
                rstd_row(rstd[:], var[:], width, tag)
                nc.vector.tensor_mul(crow[:, 0:width], rstd[:], gnw)
                nc.vector.tensor_mul(crow[:, width : 2 * width], mean[:], gnms)
                nc.vector.tensor_mul(crow[:, width : 2 * width],
                                     crow[:, width : 2 * width],
                                     crow[:, 0:width])
                nc.vector.scalar_tensor_tensor(
                    crow[:, width : 2 * width], crow[:, width : 2 * width],
                    -1.0, gnb, op0=ALU.mult, op1=ALU.add,
                )

            def emit_mlp_relu(li, b):
                """MLP matmuls + Act relu -> bf16 SBUF ring tile."""
                pre = ps.tile([128, HH], mdt.float32, tag="pre", bufs=2)
                a_sl = attrT_sb[:, b * 128 : (b + 1) * 128]
                nc.tensor.matmul(pre[:, 0:HHH], a_sl, W_l[li][:, 0:HHH],
                                 start=True, stop=True)
                nc.tensor.matmul(pre[:, HHH:HH], a_sl, W_l[li][:, HHH:HH],
                                 start=True, stop=True)
                ew = ewpool.tile([128, HH], mdt.bfloat16, tag="ew")
                nc.scalar.activation(ew[:], pre[:], AF.Relu)
                return ew

            def emit_mlp_raw(li, b):
                pre = ps.tile([128, HH], mdt.float32, tag="pre", bufs=2)
                a_sl = attrT_sb[:, b * 128 : (b + 1) * 128]
                nc.tensor.matmul(pre[:, 0:HHH], a_sl, W_l[li][:, 0:HHH],
                                 start=True, stop=True)
                nc.tensor.matmul(pre[:, HHH:HH], a_sl, W_l[li][:, HHH:HH],
                                 start=True, stop=True)
                return pre

            def path_for(li, b):
                # A: Act relu + DVE mult; D: DVE fused stt from PSUM
                return ("A", "A", "A", "D")[b % 4]

            pending = {}
            hcur, hnxt = hA, hB

            for li in range(L):
                hsrc_v = hsrc0_sb if li == 0 else hsrc_sb
                if li > 0:
                    htable = htable_l[li]
                    for b in range(B):
                        nc.gpsimd.indirect_dma_start(
                            out=hsrc_sb[:, b, :],
                            out_offset=None,
                            in_=htable[:],
                            in_offset=bass.IndirectOffsetOnAxis(
                                ap=idx_sb[:, b : b + 1], axis=0
                            ),
                        )

                # ---- edge phase (wrapup = i-reduce only; agg double-buffered)
                b = 0
                for w in range(nwin):
                    aggI = ps.tile([128, HH], mdt.float32, tag="agg", bufs=2)
                    for j in range(bw[w]):
                        path = path_for(li, b)
                        first, last = j == 0, j == bw[w] - 1
                        h_bc = (hsrc_v[:, b, :].unsqueeze(1)
                                .broadcast_to([128, H, H]))
                        tmp = tmppool.tile([128, HH], mdt.bfloat16, tag="tmp")
                        tmp_v = tmp[:].rearrange("p (o i) -> p o i", o=H, i=H)
                        if path == "D":
                            pre = emit_mlp_raw(li, b)
                            nc.vector.scalar_tensor_tensor(
                                tmp_v,
                                pre[:].rearrange("p (o i) -> p o i", o=H, i=H),
                                0.0, h_bc, op0=ALU.max, op1=ALU.mult,
                            )
                        else:
                            if (li, b) in pending:
                                ew = pending.pop((li, b))
                            else:
                                ew = emit_mlp_relu(li, b)
                            ew_v = ew[:].rearrange("p (o i) -> p o i", o=H,
                                                   i=H)
                            nc.vector.tensor_tensor(tmp_v, ew_v, h_bc,
                                                    op=ALU.mult)
                        oh = onehot_sb[:, b, :]
                        nc.tensor.matmul(aggI[:, 0:HHH], oh, tmp[:, 0:HHH],
                                         start=first, stop=last)
                        nc.tensor.matmul(aggI[:, HHH:HH], oh, tmp[:, HHH:HH],
                                         start=first, stop=last)
                        b += 1
                    nc.vector.tensor_reduce(
                        conv_sb[:, w, :],
                        aggI[:].rearrange("p (o i) -> p o i", o=H, i=H),
                        axis=mybir.AxisListType.X, op=ALU.add,
                    )

                # ---- node pass 1 (deferred, light): transposes + root +
                # stats via PSUM-accumulated masked-ones matmul chain
                rt_all = ps.tile([128, nub * H], mdt.float32, tag="pre",
                                 bufs=2)
                for w in range(nwin):
                    tp = ps.tile([H, 128], mdt.bfloat16, tag="agg", bufs=2)
                    nc.tensor.transpose(tp[:], hcur[:, w, :], ident_sb[:])
                    nc.scalar.activation(catT_all[0:H, w, :], tp[:], AF.Copy)
                    nc.tensor.matmul(rt_all[:, w * H : (w + 1) * H],
                                     catT_all[:, w, :], root_l[li][:],
                                     start=True, stop=True)
                nc.vector.tensor_add(
                    conv_sb[:].rearrange("p u f -> p (u f)"),
                    conv_sb[:].rearrange("p u f -> p (u f)"), rt_all[:])
                nc.vector.tensor_mul(st_sb[:, :, 0:H], conv_sb[:],
                                     conv_sb[:])
                nc.vector.tensor_copy(st_sb[:, :, H : 2 * H], conv_sb[:])
                smm = ps.tile([1, 2 * H], mdt.float32, tag="pre", bufs=2)
                for w in range(nwin):
                    nc.tensor.matmul(smm[:], mask_sb[:, w : w + 1],
                                     st_sb[:, w, :], start=(w == 0),
                                     stop=(w == nwin - 1))
                nc.scalar.activation(stats_sb[:], smm[:], AF.Copy)

                # ---- stats AllReduce
                nc.sync.dma_start(st_in[:], stats_sb[:])
                st_out = st_out_l[li]
                nc.gpsimd.collective_compute(
                    "AllReduce", ALU.add, replica_groups=rg,
                    ins=[st_in.opt()], outs=[st_out.opt()],
                )

                # ---- next-layer MLP runahead (fills the AllReduce window)
                if li + 1 < L:
                    for rb in range(RUNA):
                        if path_for(li + 1, rb) != "D":
                            pending[(li + 1, rb)] = emit_mlp_relu(li + 1, rb)

                srow2 = rpool.tile([1, 2 * H], mdt.float32, tag="srow")
                nc.sync.dma_start(srow2[:], st_out[:])

                # ---- C/D rows + broadcast
                crow = rpool.tile([1, 2 * H], mdt.float32, tag="cdrow")
                cd_rows(crow, srow2, gn_l[li][:, 0:H], gn_l[li][:, H : 2 * H],
                        gn_l[li][:, 2 * H : 3 * H], H, "nrow")
                cd_ps = ps.tile([128, 2 * H], mdt.float32, tag="pre", bufs=2)
                nc.tensor.matmul(cd_ps[:], onesr_sb[:], crow[:], start=True,
                                 stop=True)
                nc.scalar.activation(cd_sb[:], cd_ps[:], AF.Copy)

                # ---- node pass 2 (batched)
                nc.vector.tensor_tensor(
                    t1_all[:], conv_sb[:],
                    cd_sb[:, 0:H].unsqueeze(1).broadcast_to([128, nub, H]),
                    op=ALU.mult)
                nc.vector.tensor_tensor(
                    t1_all[:], t1_all[:],
                    cd_sb[:, H : 2 * H].unsqueeze(1)
                    .broadcast_to([128, nub, H]),
                    op=ALU.add)
                nc.vector.scalar_tensor_tensor(
                    hc_all[:], t1_all[:], 0.0, hcur[:],
                    op0=ALU.max, op1=ALU.add)
                for u in range(nub):
                    tp2 = ps.tile([H, 128], mdt.bfloat16, tag="agg", bufs=2)
                    nc.tensor.transpose(tp2[:], hc_all[:, u, :], ident_sb[:])
                    nc.scalar.activation(catT_all[H : 2 * H, u, :], tp2[:],
                                         AF.Copy)
                    tr = ps.tile([128, H], mdt.float32, tag="agg", bufs=2)
                    nc.tensor.matmul(tr[:], catT_all[:, u, :], trans_l[li][:],
                                     start=True, stop=True)
                    nc.scalar.activation(hnxt[:, u, :], tr[:], AF.Relu)

                if li + 1 < L:
                    nc.sync.dma_start(hstage_v, hnxt[:])
                    nc.gpsimd.collective_compute(
                        "AllGather", ALU.bypass, replica_groups=rg,
                        ins=[hstage_dram.opt()], outs=[htable_l[li + 1].opt()],
                    )

                hcur, hnxt = hnxt, hcur

            # ============ final (fused) ============
            fsmm = ps.tile([1, 2 * T], mdt.float32, tag="pre", bufs=2)
            for u in range(nub):
                tp3 = ps.tile([H, 128], mdt.bfloat16, tag="agg", bufs=2)
                nc.tensor.transpose(tp3[:], hcur[:, u, :], ident_sb[:])
                nc.scalar.activation(catT_all[0:H, u, :], tp3[:], AF.Copy)
                f_ps = ps.tile([128, T], mdt.float32, tag="agg", bufs=2)
                nc.tensor.matmul(f_ps[:], catT_all[:, u, :], final_sb[:],
                                 start=True, stop=True)
                nc.scalar.activation(fo_sb[:, u, :], f_ps[:], AF.Copy)
                nc.vector.tensor_mul(fst_sb[:, u, 0:T], fo_sb[:, u, :],
                                     fo_sb[:, u, :])
                nc.vector.tensor_copy(fst_sb[:, u, T : 2 * T], fo_sb[:, u, :])
                nc.tensor.matmul(fsmm[:], mask_sb[:, u : u + 1],
                                 fst_sb[:, u, :], start=(u == 0),
                                 stop=(u == nub - 1))
            nc.scalar.activation(fstats_sb[:], fsmm[:], AF.Copy)

            nc.sync.dma_start(fst_in[:], fstats_sb[:])
            nc.gpsimd.collective_compute(
                "AllReduce", ALU.add, replica_groups=rg,
                ins=[fst_in.opt()], outs=[fst_out.opt()],
            )
            fsrow2 = rpool.tile([1, 2 * T], mdt.float32, tag="fsrow")
            nc.sync.dma_start(fsrow2[:], fst_out[:])

            fcrow = rpool.tile([1, 2 * T], mdt.float32, tag="fcdrow")
            cd_rows(fcrow, fsrow2, fgn_sb[:, 0:T], fgn_sb[:, T : 2 * T],
                    fgn_sb[:, 2 * T : 3 * T], T, "frow")
            fcd_ps = ps.tile([128, 2 * T], mdt.float32, tag="pre", bufs=2)
            nc.tensor.matmul(fcd_ps[:], onesr_sb[:], fcrow[:], start=True,
                             stop=True)
            nc.scalar.activation(fcd_sb[:], fcd_ps[:], AF.Copy)

            nc.vector.tensor_tensor(
                y_sb[:], fo_sb[:],
                fcd_sb[:, 0:T].unsqueeze(1).broadcast_to([128, nub, T]),
                op=ALU.mult)
            nc.vector.tensor_tensor(
                y_sb[:], y_sb[:],
                fcd_sb[:, T : 2 * T].unsqueeze(1).broadcast_to([128, nub, T]),
                op=ALU.add)
            nc.vector.tensor_scalar_max(y_sb[:], y_sb[:], 0.0)
            out_v = out_dram.rearrange("(u p) f -> p u f", p=128)
            nc.sync.dma_start(out_v, y_sb[:])

    nc.compile()
    return nc


# ------------------------------------------------------------------ driver

_CACHE = {}


def kernel(**inputs) -> np.ndarray:
    in_maps, s, node_at_pos = prep_inputs(inputs)
    key = (s["N"], s["E"], s["B"], s["block_win"])
    if key not in _CACHE:
        _CACHE[key] = build_program(s)
    nc = _CACHE[key]

    from concourse.bass_utils import run_bass_kernel_spmd

    res = run_bass_kernel_spmd(nc, in_maps, core_ids=list(range(NCORES)))
    shard, T, N = s["shard"], s["T"], s["N"]
    out = np.empty((N, T), np.float32)
    for c in range(NCORES):
        rows = res.results[c]["out"]
        vmask = node_at_pos[c] >= 0
        out[c * shard + node_at_pos[c, vmask]] = rows[vmask]
    return out.astype(np.float32)


# revision 40
# speedup vs baseline: 1.5155x; 1.0395x over previous
"""DJMGNN (NNConv/GraphNorm GNN) Trainium2 kernel, 8-core SPMD. v2.

Sharding: nodes range-sharded N/8 per core, then PERMUTED within each shard so
every 128-node dst window holds <=512 edges (uniform 4 blocks/window, B=80
vs 98 unbalanced). Edges assigned to the core owning their dst node.

Per layer:
  - h table in DRAM; per-block indirect-DMA gather of h[src] (Pool engine).
    Layer 0 needs no table/gather: h0 = x@W is computed on HOST and shipped
    pre-gathered (hsrc0) plus the local shard (h0_own).
  - edge MLP on PE (attrT stationary, 2x512-col matmuls into one 2-bank PSUM
    tile) -> Act relu -> bf16 SBUF ring.
  - relu*h mult on DVE (bf16 2x) or Pool (layer 0), o-major broadcast AP.
  - one-hot scatter-matmul accumulating aggI[u,(o,i)] per 128-dst window.
  - interleaved node pass 1 at each window close: DVE strided i-reduce,
    root matmul (catT stationary), conv, stats via masked-ones matmul.
  - stats AllReduce; next-layer MLP runahead emitted before it to hide
    collective latency; batched pass 2; AllGather of the new h shard.
  - final transition + GraphNorm fused into layer-2 pass 2.
"""

import sys

if "/opt/trn_rl_repo" not in sys.path:
    sys.path.insert(0, "/opt/trn_rl_repo")

import numpy as np
import ml_dtypes

import concourse.bass as bass
import concourse.bacc as bacc
import concourse.mybir as mybir
import concourse.tile as tile

mdt = mybir.dt
AF = mybir.ActivationFunctionType
ALU = mybir.AluOpType

NCORES = 8
EPS = 1e-5
RUNA = 12  # next-layer MLP runahead blocks (hide stats-AllReduce latency)


# ---------------------------------------------------------------- host prep


def _balance_windows(deg, nwin, cap_n=128):
    """Assign local nodes to nwin windows, balancing edge load (greedy LPT)."""
    order = np.argsort(-deg, kind="stable")
    wload = np.zeros(nwin, dtype=np.int64)
    wn = np.zeros(nwin, dtype=np.int64)
    assign = np.full(deg.shape[0], -1, dtype=np.int64)
    for v in order:
        cand = np.where(wn < cap_n)[0]
        w = cand[np.argmin(wload[cand])]
        assign[v] = w
        wload[w] += deg[v]
        wn[w] += 1
    return assign, wload, wn


def prep_inputs(inputs):
    x = np.asarray(inputs["x"], np.float32)
    edge_attr = np.asarray(inputs["edge_attr"], np.float32)
    edge_index = np.asarray(inputs["edge_index"])
    N, IN = x.shape
    E, EA = edge_attr.shape
    H = np.asarray(inputs["init_W"]).shape[1]
    L = np.asarray(inputs["edge_mlp_W"]).shape[0]
    T = np.asarray(inputs["final_W"]).shape[1]
    shard = N // NCORES
    nwin = (shard + 127) // 128
    shard_pad = nwin * 128
    src = edge_index[0].astype(np.int64)
    dst = edge_index[1].astype(np.int64)
    owner = dst // shard
    dst_local = dst - owner * shard

    # per-core window assignment (node permutation)
    assigns, wns = [], []
    pos_of_node = np.empty((NCORES, shard), np.int64)  # node -> padded position
    node_at_pos = np.full((NCORES, shard_pad), -1, np.int64)
    cnt = np.zeros((NCORES, nwin), np.int64)
    for c in range(NCORES):
        dl = dst_local[owner == c]
        deg = np.bincount(dl, minlength=shard)
        assign, wload, wn = _balance_windows(deg, nwin)
        assigns.append(assign)
        wns.append(wn)
        cnt[c] = wload
        row_next = np.zeros(nwin, np.int64)
        for v in range(shard):
            w = assign[v]
            p = w * 128 + row_next[w]
            row_next[w] += 1
            pos_of_node[c, v] = p
            node_at_pos[c, p] = v

    bw = np.maximum((cnt + 127) // 128, 1).max(axis=0)  # blocks per window
    block_win = []
    for w in range(nwin):
        block_win += [w] * int(bw[w])
    B = len(block_win)

    # slot assignment: edges of (core, window) packed into that window's blocks
    eslot = np.full((NCORES, B, 128), -1, dtype=np.int64)
    wb0 = np.concatenate([[0], np.cumsum(bw)])  # first block of window w
    for c in range(NCORES):
        ec = np.where(owner == c)[0]
        wids = assigns[c][dst_local[ec]]
        for w in range(nwin):
            es = ec[wids == w]
            b0 = wb0[w]
            for j, e in enumerate(es):
                eslot[c, b0 + j // 128, j % 128] = e

    # host-side h0 (layer-0 table): h0 = x @ init_W + init_b
    h0 = x @ np.asarray(inputs["init_W"], np.float32) + np.asarray(
        inputs["init_b"], np.float32
    )

    # o-major reorder of edge MLP weights: col j = i*H + o -> o*H + i
    Wm = np.asarray(inputs["edge_mlp_W"], np.float32).reshape(L, EA, H, H)
    Wm = Wm.transpose(0, 1, 3, 2).reshape(L, EA, H * H)
    bm = np.asarray(inputs["edge_mlp_b"], np.float32).reshape(L, H, H)
    bm = bm.transpose(0, 2, 1).reshape(L, H * H)
    W_aug = np.concatenate([Wm, bm[:, None, :]], axis=1)  # [L, EA+1, H*H]

    rootW = np.asarray(inputs["root_W"], np.float32)
    root_aug = np.zeros((L, 2 * H + 1, H), np.float32)
    root_aug[:, :H, :] = rootW
    root_aug[:, 2 * H, :] = np.asarray(inputs["root_b"], np.float32)

    trans_aug = np.concatenate(
        [np.asarray(inputs["trans_W"], np.float32),
         np.asarray(inputs["trans_b"], np.float32)[:, None, :]], axis=1
    )  # [L, 2H+1, H]

    final_aug = np.zeros((2 * H + 1, T), np.float32)
    final_aug[:H, :] = np.asarray(inputs["final_W"], np.float32)
    final_aug[2 * H, :] = np.asarray(inputs["final_b"], np.float32)

    gn = np.concatenate(
        [np.asarray(inputs["gn_w"], np.float32),
         np.asarray(inputs["gn_b"], np.float32),
         np.asarray(inputs["gn_ms"], np.float32)], axis=1
    )[:, None, :]  # [L, 1, 3H]
    fgn = np.concatenate(
        [np.asarray(inputs["fgn_w"], np.float32),
         np.asarray(inputs["fgn_b"], np.float32),
         np.asarray(inputs["fgn_ms"], np.float32)], axis=0
    )[None, :]  # [1, 3T]

    iota = np.broadcast_to(np.arange(128, dtype=np.float32), (128, 128))
    ident = np.eye(128, dtype=np.float32)
    ones_row = np.ones((1, 128), np.float32)

    in_maps = []
    for c in range(NCORES):
        es = eslot[c]
        valid = es >= 0
        esc = np.where(valid, es, 0)
        flat = esc.reshape(-1)
        vflat = valid.reshape(-1)

        attrT_aug = np.zeros((EA + 1, B * 128), np.float32)
        attrT_aug[:EA, :] = edge_attr[flat].T * vflat
        attrT_aug[EA, :] = vflat.astype(np.float32)

        sg = src[flat]
        gidx = (sg // shard) * shard_pad + pos_of_node[sg // shard, sg % shard]
        gidx = np.where(vflat, gidx, 0).astype(np.int32)
        src_gidx = gidx.reshape(B, 128).T.copy()

        # pre-gathered layer-0 h[src] (pad slots zeroed)
        hsrc0 = (h0[sg] * vflat[:, None]).reshape(B, 128, H).transpose(1, 0, 2)

        wl = assigns[c][dst_local[flat]]
        dr = np.where(vflat,
                      pos_of_node[c, dst_local[flat]] - wl * 128, -1.0)
        dst_rel = dr.astype(np.float32).reshape(B, 128).T
        # host-built one-hot scatter blocks [128 slot, B, 128 dstrow]
        onehot = (np.arange(128, dtype=np.float32)[None, None, :]
                  == dst_rel[:, :, None])

        # own shard h0 in permuted layout [128, nwin, H]
        h0_own = np.zeros((shard_pad, H), np.float32)
        vmask = node_at_pos[c] >= 0
        h0_own[vmask] = h0[c * shard + node_at_pos[c, vmask]]
        h0_own = h0_own.reshape(nwin, 128, H).transpose(1, 0, 2)

        mask = (np.arange(128)[:, None] < wns[c][None, :]).astype(np.float32)

        in_maps.append(
            {
                "attrT_aug": np.ascontiguousarray(attrT_aug).astype(ml_dtypes.bfloat16),
                "src_gidx": np.ascontiguousarray(src_gidx),
                "onehot": np.ascontiguousarray(onehot).astype(ml_dtypes.bfloat16),
                "hsrc0": np.ascontiguousarray(hsrc0).astype(ml_dtypes.bfloat16),
                "h0_own": np.ascontiguousarray(h0_own).astype(ml_dtypes.bfloat16),
                "mask": np.ascontiguousarray(mask),
                "W_aug": W_aug.astype(ml_dtypes.bfloat16),
                "root_aug": root_aug.astype(ml_dtypes.bfloat16),
                "trans_aug": trans_aug.astype(ml_dtypes.bfloat16),
                "final_aug": final_aug.astype(ml_dtypes.bfloat16),
                "gn": gn,
                "fgn": fgn,
                "ident": ident.astype(ml_dtypes.bfloat16),
                "ones_row": ones_row,
            }
        )

    shapes = dict(
        N=N, E=E, IN=IN, H=H, EA=EA, T=T, L=L, shard=shard,
        shard_pad=shard_pad, nub=nwin, B=B, block_win=tuple(block_win),
        bw=tuple(int(v) for v in bw), nwin=nwin,
    )
    perms = node_at_pos  # for output unpermute
    return in_maps, shapes, perms


# ------------------------------------------------------------- device build


def build_program(s):
    H, EA, T, L = s["H"], s["EA"], s["T"], s["L"]
    B, nub, nwin = s["B"], s["nub"], s["nwin"]
    shard_pad = s["shard_pad"]
    block_win = s["block_win"]
    bw = s["bw"]
    HH = H * H
    HHH = HH // 2
    n_total = shard_pad * NCORES
    n_real = s["N"]

    nc = bacc.Bacc("TRN2", target_bir_lowering=False, debug=False,
                   enable_asserts=False, num_devices=NCORES)

    def din(name, shape, dtype=mdt.float32):
        return nc.dram_tensor(name, shape, dtype, kind="ExternalInput").ap()

    attrT = din("attrT_aug", [EA + 1, B * 128], mdt.bfloat16)
    src_gidx = din("src_gidx", [128, B], mdt.int32)
    onehot_in = din("onehot", [128, B, 128], mdt.bfloat16)
    hsrc0_in = din("hsrc0", [128, B, H], mdt.bfloat16)
    h0_own_in = din("h0_own", [128, nub, H], mdt.bfloat16)
    mask_in = din("mask", [128, nub])
    W_in = din("W_aug", [L, EA + 1, HH], mdt.bfloat16)
    root_in = din("root_aug", [L, 2 * H + 1, H], mdt.bfloat16)
    trans_in = din("trans_aug", [L, 2 * H + 1, H], mdt.bfloat16)
    final_in = din("final_aug", [2 * H + 1, T], mdt.bfloat16)
    gn_in = din("gn", [L, 1, 3 * H])
    fgn_in = din("fgn", [1, 3 * T])
    ident_in = din("ident", [128, 128], mdt.bfloat16)
    ones_in = din("ones_row", [1, 128])

    out_dram = nc.dram_tensor("out", [shard_pad, T], mdt.float32,
                              kind="ExternalOutput").ap()

    rg = [list(range(NCORES))]

    with tile.TileContext(nc) as tc:
        with (
            tc.tile_pool(name="const", bufs=1) as cpool,
            tc.tile_pool(name="hbuf", bufs=1) as hpool,
            tc.tile_pool(name="ew", bufs=RUNA) as ewpool,
            tc.tile_pool(name="tmp", bufs=4) as tmppool,
            tc.tile_pool(name="rows", bufs=10) as rpool,
            tc.tile_pool(name="ps", bufs=1, space="PSUM") as ps,
            tc.tile_pool(name="dram", bufs=1, space="DRAM") as dram,
        ):
            def load(pool, shape, ap, dtype=mdt.float32, tag=None):
                t = pool.tile(shape, dtype, tag=tag)
                nc.sync.dma_start(t[:], ap)
                return t

            # split big startup DMAs so early blocks land first
            SPL = RUNA * 128
            attrT_sb = cpool.tile([EA + 1, B * 128], mdt.bfloat16, tag="attrT")
            nc.sync.dma_start(attrT_sb[:, 0:SPL], attrT[:, 0:SPL])
            nc.sync.dma_start(attrT_sb[:, SPL:], attrT[:, SPL:])
            idx_sb = load(cpool, [128, B], src_gidx[:], mdt.int32, tag="sidx")
            onehot_sb = cpool.tile([128, B, 128], mdt.bfloat16, tag="onehot")
            nc.sync.dma_start(onehot_sb[:, 0:RUNA, :], onehot_in[:, 0:RUNA, :])
            nc.sync.dma_start(onehot_sb[:, RUNA:, :], onehot_in[:, RUNA:, :])
            hsrc0_sb = cpool.tile([128, B, H], mdt.bfloat16, tag="hsrc0")
            nc.sync.dma_start(hsrc0_sb[:, 0:RUNA, :], hsrc0_in[:, 0:RUNA, :])
            nc.sync.dma_start(hsrc0_sb[:, RUNA:, :], hsrc0_in[:, RUNA:, :])
            mask_sb = load(cpool, [128, nub], mask_in[:], tag="mask")
            final_sb = load(cpool, [2 * H + 1, T], final_in[:], mdt.bfloat16,
                            tag="finalw")
            ident_sb = load(cpool, [128, 128], ident_in[:], mdt.bfloat16,
                            tag="ident")
            onesr_sb = load(cpool, [1, 128], ones_in[:], tag="onesr")
            fgn_sb = load(cpool, [1, 3 * T], fgn_in[:], tag="fgn")
            W_l = [load(cpool, [EA + 1, HH], W_in[li], mdt.bfloat16,
                        tag=f"W{li}") for li in range(L)]
            root_l = [load(cpool, [2 * H + 1, H], root_in[li], mdt.bfloat16,
                           tag=f"rw{li}") for li in range(L)]
            trans_l = [load(cpool, [2 * H + 1, H], trans_in[li], mdt.bfloat16,
                            tag=f"tw{li}") for li in range(L)]
            gn_l = [load(cpool, [1, 3 * H], gn_in[li], tag=f"gn{li}")
                    for li in range(L)]

            # ---- persistent tiles
            hA = hpool.tile([128, nub, H], mdt.bfloat16)
            hB = hpool.tile([128, nub, H], mdt.bfloat16)
            hc_all = hpool.tile([128, nub, H], mdt.bfloat16)
            t1_all = hpool.tile([128, nub, H], mdt.float32)
            conv_sb = hpool.tile([128, nub, H], mdt.float32)
            st_sb = hpool.tile([128, nub, 2 * H], mdt.float32)
            fst_sb = hpool.tile([128, nub, 2 * T], mdt.float32)
            hsrc_sb = hpool.tile([128, B, H], mdt.bfloat16)
            catT_all = hpool.tile([2 * H + 1, nub, 128], mdt.bfloat16)
            fo_sb = hpool.tile([128, nub, T], mdt.float32)
            y_sb = hpool.tile([128, nub, T], mdt.float32)
            cd_sb = hpool.tile([128, 2 * H], mdt.float32)
            fcd_sb = hpool.tile([128, 2 * T], mdt.float32)
            stats_sb = hpool.tile([1, 2 * H], mdt.float32)
            fstats_sb = hpool.tile([1, 2 * T], mdt.float32)

            nc.vector.memset(catT_all[H : 2 * H, :, :], 0.0)
            nc.vector.memset(catT_all[2 * H : 2 * H + 1, :, :], 1.0)
            nc.sync.dma_start(hA[:], h0_own_in[:])

            # warmup collective: absorbs first-collective latency during
            # the layer-0 edge phase (result unused)
            warm_in = dram.tile([1, 8], mdt.float32)
            warm_out = dram.tile([1, 8], mdt.float32, addr_space="Shared")
            wrow = rpool.tile([1, 8], mdt.float32, tag="warm")
            nc.vector.memset(wrow[:], 0.0)
            nc.sync.dma_start(warm_in[:], wrow[:])
            nc.gpsimd.collective_compute(
                "AllReduce", ALU.add, replica_groups=rg,
                ins=[warm_in.opt()], outs=[warm_out.opt()],
            )

            hstage_dram = dram.tile([shard_pad, H], mdt.bfloat16)
            htable_l = [None] + [
                dram.tile([n_total, H], mdt.bfloat16, addr_space="Shared",
                          tag=f"htable{li}", name=f"htable{li}")
                for li in range(1, L)
            ]
            st_in = dram.tile([1, 2 * H], mdt.float32)
            st_out_l = [dram.tile([1, 2 * H], mdt.float32, addr_space="Shared",
                                  tag=f"stout{li}", name=f"stout{li}")
                        for li in range(L)]
            fst_in = dram.tile([1, 2 * T], mdt.float32)
            fst_out = dram.tile([1, 2 * T], mdt.float32, addr_space="Shared")

            hstage_v = hstage_dram[:].rearrange("(u p) f -> p u f", p=128)

            def rstd_row(dstrow, varrow, width, tag):
                """dstrow = 1/sqrt(varrow+EPS) via reciprocal+sqrt+Newton."""
                ve = rpool.tile([1, width], mdt.float32, tag=tag)
                nc.vector.tensor_scalar_add(ve[:], varrow, EPS)
                r2 = rpool.tile([1, width], mdt.float32, tag=tag)
                nc.vector.reciprocal(r2[:], ve[:])
                r0 = rpool.tile([1, width], mdt.float32, tag=tag)
                nc.scalar.activation(r0[:], r2[:], AF.Sqrt)
                t0 = rpool.tile([1, width], mdt.float32, tag=tag)
                nc.vector.tensor_mul(t0[:], r0[:], r0[:])
                nc.vector.tensor_mul(t0[:], t0[:], ve[:])
                nc.vector.scalar_tensor_tensor(
                    t0[:], t0[:], -0.5, r0[:], op0=ALU.mult, op1=ALU.mult
                )
                nc.vector.scalar_tensor_tensor(
                    dstrow, r0[:], 1.5, t0[:], op0=ALU.mult, op1=ALU.add
                )

            def cd_rows(crow, srow, gnw, gnb, gnms, width, tag):
                """crow[0:w] = C = rstd*w ; crow[w:2w] = D = b - ms*mean*C."""
                mean = rpool.tile([1, width], mdt.float32, tag=tag)
                nc.vector.tensor_scalar_mul(mean[:], srow[:, width : 2 * width],
                                            1.0 / n_real)
                msq = rpool.tile([1, width], mdt.float32, tag=tag)
                nc.vector.tensor_scalar_mul(msq[:], srow[:, 0:width],
                                            1.0 / n_real)
                mm = rpool.tile([1, width], mdt.float32, tag=tag)
                nc.vector.tensor_mul(mm[:], mean[:], mean[:])
                nc.vector.tensor_mul(mm[:], mm[:], gnms)
                co = rpool.tile([1, width], mdt.float32, tag=tag)
                nc.vector.tensor_scalar(co[:], gnms, -1.0, 2.0, op0=ALU.mult,
                                        op1=ALU.add)
                nc.vector.tensor_mul(mm[:], mm[:], co[:])
                var = rpool.tile([1, width], mdt.float32, tag=tag)
                nc.vector.tensor_sub(var[:], msq[:], mm[:])
                rstd = rpool.tile([1, width], mdt.float32, tag=tag)
                rstd_row(rstd[:], var[:], width, tag)
                nc.vector.tensor_mul(crow[:, 0:width], rstd[:], gnw)
                nc.vector.tensor_mul(crow[:, width : 2 * width], mean[:], gnms)
                nc.vector.tensor_mul(crow[:, width : 2 * width],
                                     crow[:, width : 2 * width],
                                     crow[:, 0:width])
                nc.vector.scalar_tensor_tensor(
                    crow[:, width : 2 * width], crow[:, width : 2 * width],
                    -1.0, gnb, op0=ALU.mult, op1=ALU.add,
                )

            def emit_mlp_raw(li, b):
                """MLP matmul pair -> fp32 PSUM tile."""
                pre = ps.tile([128, HH], mdt.float32, tag="pre", bufs=2)
                a_sl = attrT_sb[:, b * 128 : (b + 1) * 128]
                nc.tensor.matmul(pre[:, 0:HHH], a_sl, W_l[li][:, 0:HHH],
                                 start=True, stop=True)
                nc.tensor.matmul(pre[:, HHH:HH], a_sl, W_l[li][:, HHH:HH],
                                 start=True, stop=True)
                return pre

            def emit_mlp_relu(li, b):
                """MLP matmuls + Act relu -> bf16 SBUF ring tile."""
                pre = emit_mlp_raw(li, b)
                ew = ewpool.tile([128, HH], mdt.bfloat16, tag="ew")
                nc.scalar.activation(ew[:], pre[:], AF.Relu)
                return ew

            def path_for(li, b):
                # A: Act relu + DVE mult; D: DVE fused stt from PSUM
                return ("A", "A", "A", "D")[b % 4]

            pending = {}
            hcur, hnxt = hA, hB

            for li in range(L):
                hsrc_v = hsrc0_sb if li == 0 else hsrc_sb
                if li > 0:
                    htable = htable_l[li]
                    for b in range(B):
                        nc.gpsimd.indirect_dma_start(
                            out=hsrc_sb[:, b, :],
                            out_offset=None,
                            in_=htable[:],
                            in_offset=bass.IndirectOffsetOnAxis(
                                ap=idx_sb[:, b : b + 1], axis=0
                            ),
                        )

                # ---- edge phase (wrapup = i-reduce only; agg double-buffered)
                b = 0
                for w in range(nwin):
                    aggI = ps.tile([128, HH], mdt.float32, tag="agg", bufs=2)
                    for j in range(bw[w]):
                        path = path_for(li, b)
                        first, last = j == 0, j == bw[w] - 1
                        h_bc = (hsrc_v[:, b, :].unsqueeze(1)
                                .broadcast_to([128, H, H]))
                        tmp = tmppool.tile([128, HH], mdt.bfloat16, tag="tmp")
                        tmp_v = tmp[:].rearrange("p (o i) -> p o i", o=H, i=H)
                        if path == "D":
                            pre = emit_mlp_raw(li, b)
                            nc.vector.scalar_tensor_tensor(
                                tmp_v,
                                pre[:].rearrange("p (o i) -> p o i", o=H, i=H),
                                0.0, h_bc, op0=ALU.max, op1=ALU.mult,
                            )
                        else:
                            if (li, b) in pending:
                                ew = pending.pop((li, b))
                            else:
                                ew = emit_mlp_relu(li, b)
                            ew_v = ew[:].rearrange("p (o i) -> p o i", o=H,
                                                   i=H)
                            nc.vector.tensor_tensor(tmp_v, ew_v, h_bc,
                                                    op=ALU.mult)
                        oh = onehot_sb[:, b, :]
                        nc.tensor.matmul(aggI[:, 0:HHH], oh, tmp[:, 0:HHH],
                                         start=first, stop=last)
                        nc.tensor.matmul(aggI[:, HHH:HH], oh, tmp[:, HHH:HH],
                                         start=first, stop=last)
                        b += 1
                    nc.vector.tensor_reduce(
                        conv_sb[:, w, :],
                        aggI[:].rearrange("p (o i) -> p o i", o=H, i=H),
                        axis=mybir.AxisListType.X, op=ALU.add,
                    )

                # ---- node pass 1 (deferred, stage-major pipelined)
                for w in range(nwin):
                    tp = ps.tile([H, 128], mdt.bfloat16, tag="agg", bufs=2)
                    nc.tensor.transpose(tp[:], hcur[:, w, :], ident_sb[:])
                    nc.scalar.activation(catT_all[0:H, w, :], tp[:], AF.Copy)
                rt_all = ps.tile([128, nub * H], mdt.float32, tag="pre",
                                 bufs=2)
                for w in range(nwin):
                    nc.tensor.matmul(rt_all[:, w * H : (w + 1) * H],
                                     catT_all[:, w, :], root_l[li][:],
                                     start=True, stop=True)
                nc.vector.tensor_add(
                    conv_sb[:].rearrange("p u f -> p (u f)"),
                    conv_sb[:].rearrange("p u f -> p (u f)"), rt_all[:])
                nc.vector.tensor_mul(st_sb[:, :, 0:H], conv_sb[:],
                                     conv_sb[:])
                nc.vector.tensor_copy(st_sb[:, :, H : 2 * H], conv_sb[:])
                smm = ps.tile([1, 2 * H], mdt.float32, tag="pre", bufs=2)
                for w in range(nwin):
                    nc.tensor.matmul(smm[:], mask_sb[:, w : w + 1],
                                     st_sb[:, w, :], start=(w == 0),
                                     stop=(w == nwin - 1))
                nc.scalar.activation(stats_sb[:], smm[:], AF.Copy)

                # ---- stats AllReduce
                nc.sync.dma_start(st_in[:], stats_sb[:])
                st_out = st_out_l[li]
                nc.gpsimd.collective_compute(
                    "AllReduce", ALU.add, replica_groups=rg,
                    ins=[st_in.opt()], outs=[st_out.opt()],
                )

                # ---- next-layer MLP runahead (fills the AllReduce window)
                if li + 1 < L:
                    for rb in range(RUNA):
                        if path_for(li + 1, rb) != "D":
                            pending[(li + 1, rb)] = emit_mlp_relu(li + 1, rb)

                srow2 = rpool.tile([1, 2 * H], mdt.float32, tag="srow")
                nc.sync.dma_start(srow2[:], st_out[:])

                # ---- C/D rows + broadcast
                crow = rpool.tile([1, 2 * H], mdt.float32, tag="cdrow")
                cd_rows(crow, srow2, gn_l[li][:, 0:H], gn_l[li][:, H : 2 * H],
                        gn_l[li][:, 2 * H : 3 * H], H, "nrow")
                cd_ps = ps.tile([128, 2 * H], mdt.float32, tag="pre", bufs=2)
                nc.tensor.matmul(cd_ps[:], onesr_sb[:], crow[:], start=True,
                                 stop=True)
                nc.scalar.activation(cd_sb[:], cd_ps[:], AF.Copy)

                # ---- node pass 2 (batched)
                nc.vector.tensor_tensor(
                    t1_all[:], conv_sb[:],
                    cd_sb[:, 0:H].unsqueeze(1).broadcast_to([128, nub, H]),
                    op=ALU.mult)
                nc.vector.tensor_tensor(
                    t1_all[:], t1_all[:],
                    cd_sb[:, H : 2 * H].unsqueeze(1)
                    .broadcast_to([128, nub, H]),
                    op=ALU.add)
                nc.vector.scalar_tensor_tensor(
                    hc_all[:], t1_all[:], 0.0, hcur[:],
                    op0=ALU.max, op1=ALU.add)
                for u in range(nub):
                    tp2 = ps.tile([H, 128], mdt.bfloat16, tag="agg", bufs=2)
                    nc.tensor.transpose(tp2[:], hc_all[:, u, :], ident_sb[:])
                    nc.scalar.activation(catT_all[H : 2 * H, u, :], tp2[:],
                                         AF.Copy)
                for u in range(nub):
                    tr = ps.tile([128, H], mdt.float32, tag="pre", bufs=2)
                    nc.tensor.matmul(tr[:], catT_all[:, u, :], trans_l[li][:],
                                     start=True, stop=True)
                    nc.scalar.activation(hnxt[:, u, :], tr[:], AF.Relu)

                if li + 1 < L:
                    nc.sync.dma_start(hstage_v, hnxt[:])
                    nc.gpsimd.collective_compute(
                        "AllGather", ALU.bypass, replica_groups=rg,
                        ins=[hstage_dram.opt()], outs=[htable_l[li + 1].opt()],
                    )

                hcur, hnxt = hnxt, hcur

            # ============ final (fused, stage-major) ============
            for u in range(nub):
                tp3 = ps.tile([H, 128], mdt.bfloat16, tag="agg", bufs=2)
                nc.tensor.transpose(tp3[:], hcur[:, u, :], ident_sb[:])
                nc.scalar.activation(catT_all[0:H, u, :], tp3[:], AF.Copy)
            for u in range(nub):
                f_ps = ps.tile([128, T], mdt.float32, tag="pre", bufs=2)
                nc.tensor.matmul(f_ps[:], catT_all[:, u, :], final_sb[:],
                                 start=True, stop=True)
                nc.scalar.activation(fo_sb[:, u, :], f_ps[:], AF.Copy)
            nc.vector.tensor_mul(fst_sb[:, :, 0:T], fo_sb[:], fo_sb[:])
            nc.vector.tensor_copy(fst_sb[:, :, T : 2 * T], fo_sb[:])
            fsmm = ps.tile([1, 2 * T], mdt.float32, tag="agg", bufs=2)
            for u in range(nub):
                nc.tensor.matmul(fsmm[:], mask_sb[:, u : u + 1],
                                 fst_sb[:, u, :], start=(u == 0),
                                 stop=(u == nub - 1))
            nc.scalar.activation(fstats_sb[:], fsmm[:], AF.Copy)

            nc.sync.dma_start(fst_in[:], fstats_sb[:])
            nc.gpsimd.collective_compute(
                "AllReduce", ALU.add, replica_groups=rg,
                ins=[fst_in.opt()], outs=[fst_out.opt()],
            )
            fsrow2 = rpool.tile([1, 2 * T], mdt.float32, tag="fsrow")
            nc.sync.dma_start(fsrow2[:], fst_out[:])

            fcrow = rpool.tile([1, 2 * T], mdt.float32, tag="fcdrow")
            cd_rows(fcrow, fsrow2, fgn_sb[:, 0:T], fgn_sb[:, T : 2 * T],
                    fgn_sb[:, 2 * T : 3 * T], T, "frow")
            fcd_ps = ps.tile([128, 2 * T], mdt.float32, tag="pre", bufs=2)
            nc.tensor.matmul(fcd_ps[:], onesr_sb[:], fcrow[:], start=True,
                             stop=True)
            nc.scalar.activation(fcd_sb[:], fcd_ps[:], AF.Copy)

            nc.vector.tensor_tensor(
                y_sb[:], fo_sb[:],
                fcd_sb[:, 0:T].unsqueeze(1).broadcast_to([128, nub, T]),
                op=ALU.mult)
            nc.vector.tensor_tensor(
                y_sb[:], y_sb[:],
                fcd_sb[:, T : 2 * T].unsqueeze(1).broadcast_to([128, nub, T]),
                op=ALU.add)
            nc.vector.tensor_scalar_max(y_sb[:], y_sb[:], 0.0)
            out_v = out_dram.rearrange("(u p) f -> p u f", p=128)
            nc.sync.dma_start(out_v, y_sb[:])

    nc.compile()
    return nc


# ------------------------------------------------------------------ driver

_CACHE = {}


def kernel(**inputs) -> np.ndarray:
    in_maps, s, node_at_pos = prep_inputs(inputs)
    key = (s["N"], s["E"], s["B"], s["block_win"])
    if key not in _CACHE:
        _CACHE[key] = build_program(s)
    nc = _CACHE[key]

    from concourse.bass_utils import run_bass_kernel_spmd

    res = run_bass_kernel_spmd(nc, in_maps, core_ids=list(range(NCORES)))
    shard, T, N = s["shard"], s["T"], s["N"]
    out = np.empty((N, T), np.float32)
    for c in range(NCORES):
        rows = res.results[c]["out"]
        vmask = node_at_pos[c] >= 0
        out[c * shard + node_at_pos[c, vmask]] = rows[vmask]
    return out.astype(np.float32)


# revision 45
# speedup vs baseline: 1.6368x; 1.0801x over previous
"""DJMGNN (NNConv/GraphNorm GNN) Trainium2 kernel, 8-core SPMD. v2.

Sharding: nodes range-sharded N/8 per core, then PERMUTED within each shard so
every 128-node dst window holds <=512 edges (uniform 4 blocks/window, B=80
vs 98 unbalanced). Edges assigned to the core owning their dst node.

Per layer:
  - h table in DRAM; per-block indirect-DMA gather of h[src] (Pool engine).
    Layer 0 needs no table/gather: h0 = x@W is computed on HOST and shipped
    pre-gathered (hsrc0) plus the local shard (h0_own).
  - edge MLP on PE (attrT stationary, 2x512-col matmuls into one 2-bank PSUM
    tile) -> Act relu -> bf16 SBUF ring.
  - relu*h mult on DVE (bf16 2x) or Pool (layer 0), o-major broadcast AP.
  - one-hot scatter-matmul accumulating aggI[u,(o,i)] per 128-dst window.
  - interleaved node pass 1 at each window close: DVE strided i-reduce,
    root matmul (catT stationary), conv, stats via masked-ones matmul.
  - stats AllReduce; next-layer MLP runahead emitted before it to hide
    collective latency; batched pass 2; AllGather of the new h shard.
  - final transition + GraphNorm fused into layer-2 pass 2.
"""

import sys

if "/opt/trn_rl_repo" not in sys.path:
    sys.path.insert(0, "/opt/trn_rl_repo")

import numpy as np
import ml_dtypes

import concourse.bass as bass
import concourse.bacc as bacc
import concourse.mybir as mybir
import concourse.tile as tile

mdt = mybir.dt
AF = mybir.ActivationFunctionType
ALU = mybir.AluOpType

NCORES = 8
EPS = 1e-5
RUNA = 12  # next-layer MLP runahead blocks (hide stats-AllReduce latency)


# ---------------------------------------------------------------- host prep


def _balance_windows(deg, nwin, cap_n=128):
    """Assign local nodes to nwin windows, balancing edge load (greedy LPT)."""
    order = np.argsort(-deg, kind="stable")
    wload = np.zeros(nwin, dtype=np.int64)
    wn = np.zeros(nwin, dtype=np.int64)
    assign = np.full(deg.shape[0], -1, dtype=np.int64)
    for v in order:
        cand = np.where(wn < cap_n)[0]
        w = cand[np.argmin(wload[cand])]
        assign[v] = w
        wload[w] += deg[v]
        wn[w] += 1
    return assign, wload, wn


def prep_inputs(inputs):
    x = np.asarray(inputs["x"], np.float32)
    edge_attr = np.asarray(inputs["edge_attr"], np.float32)
    edge_index = np.asarray(inputs["edge_index"])
    N, IN = x.shape
    E, EA = edge_attr.shape
    H = np.asarray(inputs["init_W"]).shape[1]
    L = np.asarray(inputs["edge_mlp_W"]).shape[0]
    T = np.asarray(inputs["final_W"]).shape[1]
    shard = N // NCORES
    nwin = (shard + 127) // 128
    shard_pad = nwin * 128
    src = edge_index[0].astype(np.int64)
    dst = edge_index[1].astype(np.int64)
    owner = dst // shard
    dst_local = dst - owner * shard

    # per-core window assignment (node permutation)
    assigns, wns = [], []
    pos_of_node = np.empty((NCORES, shard), np.int64)  # node -> padded position
    node_at_pos = np.full((NCORES, shard_pad), -1, np.int64)
    cnt = np.zeros((NCORES, nwin), np.int64)
    for c in range(NCORES):
        dl = dst_local[owner == c]
        deg = np.bincount(dl, minlength=shard)
        assign, wload, wn = _balance_windows(deg, nwin)
        assigns.append(assign)
        wns.append(wn)
        cnt[c] = wload
        row_next = np.zeros(nwin, np.int64)
        for v in range(shard):
            w = assign[v]
            p = w * 128 + row_next[w]
            row_next[w] += 1
            pos_of_node[c, v] = p
            node_at_pos[c, p] = v

    bw = np.maximum((cnt + 127) // 128, 1).max(axis=0)  # blocks per window
    block_win = []
    for w in range(nwin):
        block_win += [w] * int(bw[w])
    B = len(block_win)

    # slot assignment: edges of (core, window) packed into that window's blocks
    eslot = np.full((NCORES, B, 128), -1, dtype=np.int64)
    wb0 = np.concatenate([[0], np.cumsum(bw)])  # first block of window w
    for c in range(NCORES):
        ec = np.where(owner == c)[0]
        wids = assigns[c][dst_local[ec]]
        for w in range(nwin):
            es = ec[wids == w]
            b0 = wb0[w]
            for j, e in enumerate(es):
                eslot[c, b0 + j // 128, j % 128] = e

    # host-side h0 (layer-0 table): h0 = x @ init_W + init_b
    h0 = x @ np.asarray(inputs["init_W"], np.float32) + np.asarray(
        inputs["init_b"], np.float32
    )

    # o-major reorder of edge MLP weights: col j = i*H + o -> o*H + i
    Wm = np.asarray(inputs["edge_mlp_W"], np.float32).reshape(L, EA, H, H)
    Wm = Wm.transpose(0, 1, 3, 2).reshape(L, EA, H * H)
    bm = np.asarray(inputs["edge_mlp_b"], np.float32).reshape(L, H, H)
    bm = bm.transpose(0, 2, 1).reshape(L, H * H)
    W_aug = np.concatenate([Wm, bm[:, None, :]], axis=1)  # [L, EA+1, H*H]

    rootW = np.asarray(inputs["root_W"], np.float32)
    root_aug = np.zeros((L, 2 * H + 1, H), np.float32)
    root_aug[:, :H, :] = rootW
    root_aug[:, 2 * H, :] = np.asarray(inputs["root_b"], np.float32)

    trans_aug = np.concatenate(
        [np.asarray(inputs["trans_W"], np.float32),
         np.asarray(inputs["trans_b"], np.float32)[:, None, :]], axis=1
    )  # [L, 2H+1, H]

    final_aug = np.zeros((2 * H + 1, T), np.float32)
    final_aug[:H, :] = np.asarray(inputs["final_W"], np.float32)
    final_aug[2 * H, :] = np.asarray(inputs["final_b"], np.float32)

    gn = np.concatenate(
        [np.asarray(inputs["gn_w"], np.float32),
         np.asarray(inputs["gn_b"], np.float32),
         np.asarray(inputs["gn_ms"], np.float32)], axis=1
    )[:, None, :]  # [L, 1, 3H]
    fgn = np.concatenate(
        [np.asarray(inputs["fgn_w"], np.float32),
         np.asarray(inputs["fgn_b"], np.float32),
         np.asarray(inputs["fgn_ms"], np.float32)], axis=0
    )[None, :]  # [1, 3T]

    iota = np.broadcast_to(np.arange(128, dtype=np.float32), (128, 128))
    ident = np.eye(128, dtype=np.float32)
    ones_row = np.ones((1, 128), np.float32)

    in_maps = []
    for c in range(NCORES):
        es = eslot[c]
        valid = es >= 0
        esc = np.where(valid, es, 0)
        flat = esc.reshape(-1)
        vflat = valid.reshape(-1)

        attrT_aug = np.zeros((EA + 1, B * 128), np.float32)
        attrT_aug[:EA, :] = edge_attr[flat].T * vflat
        attrT_aug[EA, :] = vflat.astype(np.float32)

        sg = src[flat]
        gidx = (sg // shard) * shard_pad + pos_of_node[sg // shard, sg % shard]
        gidx = np.where(vflat, gidx, 0).astype(np.int32)
        src_gidx = gidx.reshape(B, 128).T.copy()

        # pre-gathered layer-0 h[src] (pad slots zeroed)
        hsrc0 = (h0[sg] * vflat[:, None]).reshape(B, 128, H).transpose(1, 0, 2)

        wl = assigns[c][dst_local[flat]]
        dr = np.where(vflat,
                      pos_of_node[c, dst_local[flat]] - wl * 128, -1.0)
        dst_rel = dr.astype(np.float32).reshape(B, 128).T
        # host-built one-hot scatter blocks [128 slot, B, 128 dstrow]
        onehot = (np.arange(128, dtype=np.float32)[None, None, :]
                  == dst_rel[:, :, None])

        # own shard h0 in permuted layout [128, nwin, H]
        h0_own = np.zeros((shard_pad, H), np.float32)
        vmask = node_at_pos[c] >= 0
        h0_own[vmask] = h0[c * shard + node_at_pos[c, vmask]]
        h0_own = h0_own.reshape(nwin, 128, H).transpose(1, 0, 2)

        mask = (np.arange(128)[:, None] < wns[c][None, :]).astype(np.float32)

        in_maps.append(
            {
                "attrT_aug": np.ascontiguousarray(attrT_aug).astype(ml_dtypes.bfloat16),
                "src_gidx": np.ascontiguousarray(src_gidx),
                "onehot": np.ascontiguousarray(onehot).astype(ml_dtypes.bfloat16),
                "hsrc0": np.ascontiguousarray(hsrc0).astype(ml_dtypes.bfloat16),
                "h0_own": np.ascontiguousarray(h0_own).astype(ml_dtypes.bfloat16),
                "mask": np.ascontiguousarray(mask),
                "W_aug": W_aug.astype(ml_dtypes.bfloat16),
                "root_aug": root_aug.astype(ml_dtypes.bfloat16),
                "trans_aug": trans_aug.astype(ml_dtypes.bfloat16),
                "final_aug": final_aug.astype(ml_dtypes.bfloat16),
                "gn": gn,
                "fgn": fgn,
                "ident": ident.astype(ml_dtypes.bfloat16),
                "ones_row": ones_row,
            }
        )

    shapes = dict(
        N=N, E=E, IN=IN, H=H, EA=EA, T=T, L=L, shard=shard,
        shard_pad=shard_pad, nub=nwin, B=B, block_win=tuple(block_win),
        bw=tuple(int(v) for v in bw), nwin=nwin,
    )
    perms = node_at_pos  # for output unpermute
    return in_maps, shapes, perms


# ------------------------------------------------------------- device build


def build_program(s):
    H, EA, T, L = s["H"], s["EA"], s["T"], s["L"]
    B, nub, nwin = s["B"], s["nub"], s["nwin"]
    shard_pad = s["shard_pad"]
    block_win = s["block_win"]
    bw = s["bw"]
    HH = H * H
    HHH = HH // 2
    n_total = shard_pad * NCORES
    n_real = s["N"]

    nc = bacc.Bacc("TRN2", target_bir_lowering=False, debug=False,
                   enable_asserts=False, num_devices=NCORES)

    def din(name, shape, dtype=mdt.float32):
        return nc.dram_tensor(name, shape, dtype, kind="ExternalInput").ap()

    attrT = din("attrT_aug", [EA + 1, B * 128], mdt.bfloat16)
    src_gidx = din("src_gidx", [128, B], mdt.int32)
    onehot_in = din("onehot", [128, B, 128], mdt.bfloat16)
    hsrc0_in = din("hsrc0", [128, B, H], mdt.bfloat16)
    h0_own_in = din("h0_own", [128, nub, H], mdt.bfloat16)
    mask_in = din("mask", [128, nub])
    W_in = din("W_aug", [L, EA + 1, HH], mdt.bfloat16)
    root_in = din("root_aug", [L, 2 * H + 1, H], mdt.bfloat16)
    trans_in = din("trans_aug", [L, 2 * H + 1, H], mdt.bfloat16)
    final_in = din("final_aug", [2 * H + 1, T], mdt.bfloat16)
    gn_in = din("gn", [L, 1, 3 * H])
    fgn_in = din("fgn", [1, 3 * T])
    ident_in = din("ident", [128, 128], mdt.bfloat16)
    ones_in = din("ones_row", [1, 128])

    out_dram = nc.dram_tensor("out", [shard_pad, T], mdt.float32,
                              kind="ExternalOutput").ap()

    rg = [list(range(NCORES))]

    with tile.TileContext(nc) as tc:
        with (
            tc.tile_pool(name="const", bufs=1) as cpool,
            tc.tile_pool(name="hbuf", bufs=1) as hpool,
            tc.tile_pool(name="ew", bufs=RUNA) as ewpool,
            tc.tile_pool(name="tmp", bufs=4) as tmppool,
            tc.tile_pool(name="rows", bufs=10) as rpool,
            tc.tile_pool(name="ps", bufs=1, space="PSUM") as ps,
            tc.tile_pool(name="dram", bufs=1, space="DRAM") as dram,
        ):
            def load(pool, shape, ap, dtype=mdt.float32, tag=None):
                t = pool.tile(shape, dtype, tag=tag)
                nc.sync.dma_start(t[:], ap)
                return t

            # split big startup DMAs so early blocks land first
            SPL = RUNA * 128
            attrT_sb = cpool.tile([EA + 1, B * 128], mdt.bfloat16, tag="attrT")
            nc.sync.dma_start(attrT_sb[:, 0:SPL], attrT[:, 0:SPL])
            nc.sync.dma_start(attrT_sb[:, SPL:], attrT[:, SPL:])
            idx_sb = load(cpool, [128, B], src_gidx[:], mdt.int32, tag="sidx")
            onehot_sb = cpool.tile([128, B, 128], mdt.bfloat16, tag="onehot")
            nc.sync.dma_start(onehot_sb[:, 0:RUNA, :], onehot_in[:, 0:RUNA, :])
            nc.sync.dma_start(onehot_sb[:, RUNA:, :], onehot_in[:, RUNA:, :])
            hsrc0_sb = cpool.tile([128, B, H], mdt.bfloat16, tag="hsrc0")
            nc.sync.dma_start(hsrc0_sb[:, 0:RUNA, :], hsrc0_in[:, 0:RUNA, :])
            nc.sync.dma_start(hsrc0_sb[:, RUNA:, :], hsrc0_in[:, RUNA:, :])
            mask_sb = load(cpool, [128, nub], mask_in[:], tag="mask")
            final_sb = load(cpool, [2 * H + 1, T], final_in[:], mdt.bfloat16,
                            tag="finalw")
            ident_sb = load(cpool, [128, 128], ident_in[:], mdt.bfloat16,
                            tag="ident")
            onesr_sb = load(cpool, [1, 128], ones_in[:], tag="onesr")
            fgn_sb = load(cpool, [1, 3 * T], fgn_in[:], tag="fgn")
            W_l = [load(cpool, [EA + 1, HH], W_in[li], mdt.bfloat16,
                        tag=f"W{li}") for li in range(L)]
            root_l = [load(cpool, [2 * H + 1, H], root_in[li], mdt.bfloat16,
                           tag=f"rw{li}") for li in range(L)]
            trans_l = [load(cpool, [2 * H + 1, H], trans_in[li], mdt.bfloat16,
                            tag=f"tw{li}") for li in range(L)]
            gn_l = [load(cpool, [1, 3 * H], gn_in[li], tag=f"gn{li}")
                    for li in range(L)]

            # ---- persistent tiles
            hA = hpool.tile([128, nub, H], mdt.bfloat16)
            hB = hpool.tile([128, nub, H], mdt.bfloat16)
            hc_all = hpool.tile([128, nub, H], mdt.bfloat16)
            t1_all = hpool.tile([128, nub, H], mdt.float32)
            conv_sb = hpool.tile([128, nub, H], mdt.float32)
            st_sb = hpool.tile([128, nub, 2 * H], mdt.float32)
            fst_sb = hpool.tile([128, nub, 2 * T], mdt.float32)
            hsrc_sb = hpool.tile([128, B, H], mdt.bfloat16)
            catT_all = hpool.tile([2 * H + 1, nub, 128], mdt.bfloat16)
            fo_sb = hpool.tile([128, nub, T], mdt.float32)
            y_sb = hpool.tile([128, nub, T], mdt.float32)
            cd_sb = hpool.tile([128, 2 * H], mdt.float32)
            fcd_sb = hpool.tile([128, 2 * T], mdt.float32)
            stats_sb = hpool.tile([1, 2 * H], mdt.float32)
            fstats_sb = hpool.tile([1, 2 * T], mdt.float32)

            nc.vector.memset(catT_all[H : 2 * H, :, :], 0.0)
            nc.vector.memset(catT_all[2 * H : 2 * H + 1, :, :], 1.0)
            nc.sync.dma_start(hA[:], h0_own_in[:])

            # warmup collective: absorbs first-collective latency during
            # the layer-0 edge phase (result unused)
            warm_in = dram.tile([1, 8], mdt.float32)
            warm_out = dram.tile([1, 8], mdt.float32, addr_space="Shared")
            wrow = rpool.tile([1, 8], mdt.float32, tag="warm")
            nc.vector.memset(wrow[:], 0.0)
            nc.sync.dma_start(warm_in[:], wrow[:])
            nc.gpsimd.collective_compute(
                "AllReduce", ALU.add, replica_groups=rg,
                ins=[warm_in.opt()], outs=[warm_out.opt()],
            )

            hstage_dram = dram.tile([shard_pad, H], mdt.bfloat16)
            htable_l = [None] + [
                dram.tile([n_total, H], mdt.bfloat16, addr_space="Shared",
                          tag=f"htable{li}", name=f"htable{li}")
                for li in range(1, L)
            ]
            st_in = dram.tile([1, 2 * H], mdt.float32)
            st_out_l = [dram.tile([1, 2 * H], mdt.float32, addr_space="Shared",
                                  tag=f"stout{li}", name=f"stout{li}")
                        for li in range(L)]
            fst_in = dram.tile([1, 2 * T], mdt.float32)
            fst_out = dram.tile([1, 2 * T], mdt.float32, addr_space="Shared")

            hstage_v = hstage_dram[:].rearrange("(u p) f -> p u f", p=128)

            def rstd_row(dstrow, varrow, width, tag):
                """dstrow = 1/sqrt(varrow+EPS) via reciprocal+sqrt+Newton."""
                ve = rpool.tile([1, width], mdt.float32, tag=tag)
                nc.vector.tensor_scalar_add(ve[:], varrow, EPS)
                r2 = rpool.tile([1, width], mdt.float32, tag=tag)
                nc.vector.reciprocal(r2[:], ve[:])
                r0 = rpool.tile([1, width], mdt.float32, tag=tag)
                nc.scalar.activation(r0[:], r2[:], AF.Sqrt)
                t0 = rpool.tile([1, width], mdt.float32, tag=tag)
                nc.vector.tensor_mul(t0[:], r0[:], r0[:])
                nc.vector.tensor_mul(t0[:], t0[:], ve[:])
                nc.vector.scalar_tensor_tensor(
                    t0[:], t0[:], -0.5, r0[:], op0=ALU.mult, op1=ALU.mult
                )
                nc.vector.scalar_tensor_tensor(
                    dstrow, r0[:], 1.5, t0[:], op0=ALU.mult, op1=ALU.add
                )

            def cd_rows(crow, srow, gnw, gnb, gnms, width, tag):
                """crow[0:w] = C = rstd*w ; crow[w:2w] = D = b - ms*mean*C."""
                mean = rpool.tile([1, width], mdt.float32, tag=tag)
                nc.vector.tensor_scalar_mul(mean[:], srow[:, width : 2 * width],
                                            1.0 / n_real)
                msq = rpool.tile([1, width], mdt.float32, tag=tag)
                nc.vector.tensor_scalar_mul(msq[:], srow[:, 0:width],
                                            1.0 / n_real)
                mm = rpool.tile([1, width], mdt.float32, tag=tag)
                nc.vector.tensor_mul(mm[:], mean[:], mean[:])
                nc.vector.tensor_mul(mm[:], mm[:], gnms)
                co = rpool.tile([1, width], mdt.float32, tag=tag)
                nc.vector.tensor_scalar(co[:], gnms, -1.0, 2.0, op0=ALU.mult,
                                        op1=ALU.add)
                nc.vector.tensor_mul(mm[:], mm[:], co[:])
                var = rpool.tile([1, width], mdt.float32, tag=tag)
                nc.vector.tensor_sub(var[:], msq[:], mm[:])
                rstd = rpool.tile([1, width], mdt.float32, tag=tag)
                rstd_row(rstd[:], var[:], width, tag)
                nc.vector.tensor_mul(crow[:, 0:width], rstd[:], gnw)
                nc.vector.tensor_mul(crow[:, width : 2 * width], mean[:], gnms)
                nc.vector.tensor_mul(crow[:, width : 2 * width],
                                     crow[:, width : 2 * width],
                                     crow[:, 0:width])
                nc.vector.scalar_tensor_tensor(
                    crow[:, width : 2 * width], crow[:, width : 2 * width],
                    -1.0, gnb, op0=ALU.mult, op1=ALU.add,
                )

            def emit_mlp_raw(li, b):
                """MLP matmul pair -> fp32 PSUM tile."""
                pre = ps.tile([128, HH], mdt.float32, tag="pre", bufs=3)
                a_sl = attrT_sb[:, b * 128 : (b + 1) * 128]
                nc.tensor.matmul(pre[:, 0:HHH], a_sl, W_l[li][:, 0:HHH],
                                 start=True, stop=True)
                nc.tensor.matmul(pre[:, HHH:HH], a_sl, W_l[li][:, HHH:HH],
                                 start=True, stop=True)
                return pre

            def emit_mlp_relu(li, b):
                """MLP matmuls + Act relu -> bf16 SBUF ring tile."""
                pre = emit_mlp_raw(li, b)
                ew = ewpool.tile([128, HH], mdt.bfloat16, tag="ew")
                nc.scalar.activation(ew[:], pre[:], AF.Relu)
                return ew

            pending = {}
            hcur, hnxt = hA, hB

            for li in range(L):
                hsrc_v = hsrc0_sb if li == 0 else hsrc_sb
                if li > 0:
                    htable = htable_l[li]
                    for b in range(B):
                        nc.gpsimd.indirect_dma_start(
                            out=hsrc_sb[:, b, :],
                            out_offset=None,
                            in_=htable[:],
                            in_offset=bass.IndirectOffsetOnAxis(
                                ap=idx_sb[:, b : b + 1], axis=0
                            ),
                        )

                # ---- edge phase: per-block i-reduce (bf16 2x) then a narrow
                # 32-col one-hot scatter into a tiny PSUM accumulator
                b = 0
                for w in range(nwin):
                    aggI = ps.tile([128, H], mdt.float32, tag="agg", bufs=2)
                    for j in range(bw[w]):
                        first, last = j == 0, j == bw[w] - 1
                        h_bc = (hsrc_v[:, b, :].unsqueeze(1)
                                .broadcast_to([128, H, H]))
                        tmp = tmppool.tile([128, HH], mdt.bfloat16, tag="tmp")
                        tmp_v = tmp[:].rearrange("p (o i) -> p o i", o=H, i=H)
                        if (li, b) in pending:
                            ew = pending.pop((li, b))
                        else:
                            ew = emit_mlp_relu(li, b)
                        ew_v = ew[:].rearrange("p (o i) -> p o i", o=H, i=H)
                        nc.vector.tensor_tensor(tmp_v, ew_v, h_bc,
                                                op=ALU.mult)
                        msg = tmppool.tile([128, H], mdt.bfloat16, tag="msg")
                        with nc.allow_low_precision(reason="msg bf16 i-sum"):
                            nc.vector.tensor_reduce(
                                msg[:], tmp_v, axis=mybir.AxisListType.X,
                                op=ALU.add)
                        nc.tensor.matmul(aggI[:], onehot_sb[:, b, :], msg[:],
                                         start=first, stop=last)
                        b += 1
                    nc.scalar.activation(conv_sb[:, w, :], aggI[:], AF.Copy)

                # ---- node pass 1 (deferred, stage-major pipelined)
                for w in range(nwin):
                    tp = ps.tile([H, 128], mdt.bfloat16, tag="agg", bufs=2)
                    nc.tensor.transpose(tp[:], hcur[:, w, :], ident_sb[:])
                    nc.scalar.activation(catT_all[0:H, w, :], tp[:], AF.Copy)
                rt_all = ps.tile([128, nub * H], mdt.float32, tag="pre",
                                 bufs=3)
                for w in range(nwin):
                    nc.tensor.matmul(rt_all[:, w * H : (w + 1) * H],
                                     catT_all[:, w, :], root_l[li][:],
                                     start=True, stop=True)
                nc.vector.tensor_add(
                    conv_sb[:].rearrange("p u f -> p (u f)"),
                    conv_sb[:].rearrange("p u f -> p (u f)"), rt_all[:])
                nc.vector.tensor_mul(st_sb[:, :, 0:H], conv_sb[:],
                                     conv_sb[:])
                nc.vector.tensor_copy(st_sb[:, :, H : 2 * H], conv_sb[:])
                smm = ps.tile([1, 2 * H], mdt.float32, tag="pre", bufs=3)
                for w in range(nwin):
                    nc.tensor.matmul(smm[:], mask_sb[:, w : w + 1],
                                     st_sb[:, w, :], start=(w == 0),
                                     stop=(w == nwin - 1))
                nc.scalar.activation(stats_sb[:], smm[:], AF.Copy)

                # ---- stats AllReduce
                nc.sync.dma_start(st_in[:], stats_sb[:])
                st_out = st_out_l[li]
                nc.gpsimd.collective_compute(
                    "AllReduce", ALU.add, replica_groups=rg,
                    ins=[st_in.opt()], outs=[st_out.opt()],
                )

                # ---- next-layer MLP runahead (fills the AllReduce window)
                if li + 1 < L:
                    for rb in range(RUNA):
                        pending[(li + 1, rb)] = emit_mlp_relu(li + 1, rb)

                srow2 = rpool.tile([1, 2 * H], mdt.float32, tag="srow")
                nc.sync.dma_start(srow2[:], st_out[:])

                # ---- C/D rows + broadcast
                crow = rpool.tile([1, 2 * H], mdt.float32, tag="cdrow")
                cd_rows(crow, srow2, gn_l[li][:, 0:H], gn_l[li][:, H : 2 * H],
                        gn_l[li][:, 2 * H : 3 * H], H, "nrow")
                cd_ps = ps.tile([128, 2 * H], mdt.float32, tag="pre", bufs=3)
                nc.tensor.matmul(cd_ps[:], onesr_sb[:], crow[:], start=True,
                                 stop=True)
                nc.scalar.activation(cd_sb[:], cd_ps[:], AF.Copy)

                # ---- node pass 2 (batched)
                nc.vector.tensor_tensor(
                    t1_all[:], conv_sb[:],
                    cd_sb[:, 0:H].unsqueeze(1).broadcast_to([128, nub, H]),
                    op=ALU.mult)
                nc.vector.tensor_tensor(
                    t1_all[:], t1_all[:],
                    cd_sb[:, H : 2 * H].unsqueeze(1)
                    .broadcast_to([128, nub, H]),
                    op=ALU.add)
                nc.vector.scalar_tensor_tensor(
                    hc_all[:], t1_all[:], 0.0, hcur[:],
                    op0=ALU.max, op1=ALU.add)
                for u in range(nub):
                    tp2 = ps.tile([H, 128], mdt.bfloat16, tag="agg", bufs=2)
                    nc.tensor.transpose(tp2[:], hc_all[:, u, :], ident_sb[:])
                    nc.scalar.activation(catT_all[H : 2 * H, u, :], tp2[:],
                                         AF.Copy)
                for u in range(nub):
                    tr = ps.tile([128, H], mdt.float32, tag="pre", bufs=3)
                    nc.tensor.matmul(tr[:], catT_all[:, u, :], trans_l[li][:],
                                     start=True, stop=True)
                    nc.scalar.activation(hnxt[:, u, :], tr[:], AF.Relu)

                if li + 1 < L:
                    nc.sync.dma_start(hstage_v, hnxt[:])
                    nc.gpsimd.collective_compute(
                        "AllGather", ALU.bypass, replica_groups=rg,
                        ins=[hstage_dram.opt()], outs=[htable_l[li + 1].opt()],
                    )

                hcur, hnxt = hnxt, hcur

            # ============ final (fused, stage-major) ============
            for u in range(nub):
                tp3 = ps.tile([H, 128], mdt.bfloat16, tag="agg", bufs=2)
                nc.tensor.transpose(tp3[:], hcur[:, u, :], ident_sb[:])
                nc.scalar.activation(catT_all[0:H, u, :], tp3[:], AF.Copy)
            for u in range(nub):
                f_ps = ps.tile([128, T], mdt.float32, tag="pre", bufs=3)
                nc.tensor.matmul(f_ps[:], catT_all[:, u, :], final_sb[:],
                                 start=True, stop=True)
                nc.scalar.activation(fo_sb[:, u, :], f_ps[:], AF.Copy)
            nc.vector.tensor_mul(fst_sb[:, :, 0:T], fo_sb[:], fo_sb[:])
            nc.vector.tensor_copy(fst_sb[:, :, T : 2 * T], fo_sb[:])
            fsmm = ps.tile([1, 2 * T], mdt.float32, tag="agg", bufs=2)
            for u in range(nub):
                nc.tensor.matmul(fsmm[:], mask_sb[:, u : u + 1],
                                 fst_sb[:, u, :], start=(u == 0),
                                 stop=(u == nub - 1))
            nc.scalar.activation(fstats_sb[:], fsmm[:], AF.Copy)

            nc.sync.dma_start(fst_in[:], fstats_sb[:])
            nc.gpsimd.collective_compute(
                "AllReduce", ALU.add, replica_groups=rg,
                ins=[fst_in.opt()], outs=[fst_out.opt()],
            )
            fsrow2 = rpool.tile([1, 2 * T], mdt.float32, tag="fsrow")
            nc.sync.dma_start(fsrow2[:], fst_out[:])

            fcrow = rpool.tile([1, 2 * T], mdt.float32, tag="fcdrow")
            cd_rows(fcrow, fsrow2, fgn_sb[:, 0:T], fgn_sb[:, T : 2 * T],
                    fgn_sb[:, 2 * T : 3 * T], T, "frow")
            fcd_ps = ps.tile([128, 2 * T], mdt.float32, tag="pre", bufs=3)
            nc.tensor.matmul(fcd_ps[:], onesr_sb[:], fcrow[:], start=True,
                             stop=True)
            nc.scalar.activation(fcd_sb[:], fcd_ps[:], AF.Copy)

            nc.vector.tensor_tensor(
                y_sb[:], fo_sb[:],
                fcd_sb[:, 0:T].unsqueeze(1).broadcast_to([128, nub, T]),
                op=ALU.mult)
            nc.vector.tensor_tensor(
                y_sb[:], y_sb[:],
                fcd_sb[:, T : 2 * T].unsqueeze(1).broadcast_to([128, nub, T]),
                op=ALU.add)
            nc.vector.tensor_scalar_max(y_sb[:], y_sb[:], 0.0)
            out_v = out_dram.rearrange("(u p) f -> p u f", p=128)
            nc.sync.dma_start(out_v, y_sb[:])

    nc.compile()
    return nc


# ------------------------------------------------------------------ driver

_CACHE = {}


def kernel(**inputs) -> np.ndarray:
    in_maps, s, node_at_pos = prep_inputs(inputs)
    key = (s["N"], s["E"], s["B"], s["block_win"])
    if key not in _CACHE:
        _CACHE[key] = build_program(s)
    nc = _CACHE[key]

    from concourse.bass_utils import run_bass_kernel_spmd

    res = run_bass_kernel_spmd(nc, in_maps, core_ids=list(range(NCORES)))
    shard, T, N = s["shard"], s["T"], s["N"]
    out = np.empty((N, T), np.float32)
    for c in range(NCORES):
        rows = res.results[c]["out"]
        vmask = node_at_pos[c] >= 0
        out[c * shard + node_at_pos[c, vmask]] = rows[vmask]
    return out.astype(np.float32)


# revision 55
# speedup vs baseline: 1.7031x; 1.0405x over previous
"""DJMGNN (NNConv/GraphNorm GNN) Trainium2 kernel, 8-core SPMD. v2.

Sharding: nodes range-sharded N/8 per core, then PERMUTED within each shard so
every 128-node dst window holds <=512 edges (uniform 4 blocks/window, B=80
vs 98 unbalanced). Edges assigned to the core owning their dst node.

Per layer:
  - h table in DRAM; per-block indirect-DMA gather of h[src] (Pool engine).
    Layer 0 needs no table/gather: h0 = x@W is computed on HOST and shipped
    pre-gathered (hsrc0) plus the local shard (h0_own).
  - edge MLP on PE (attrT stationary, 2x512-col matmuls into one 2-bank PSUM
    tile) -> Act relu -> bf16 SBUF ring.
  - relu*h mult on DVE (bf16 2x) or Pool (layer 0), o-major broadcast AP.
  - one-hot scatter-matmul accumulating aggI[u,(o,i)] per 128-dst window.
  - interleaved node pass 1 at each window close: DVE strided i-reduce,
    root matmul (catT stationary), conv, stats via masked-ones matmul.
  - stats AllReduce; next-layer MLP runahead emitted before it to hide
    collective latency; batched pass 2; AllGather of the new h shard.
  - final transition + GraphNorm fused into layer-2 pass 2.
"""

import sys

if "/opt/trn_rl_repo" not in sys.path:
    sys.path.insert(0, "/opt/trn_rl_repo")

import numpy as np
import ml_dtypes

import concourse.bass as bass
import concourse.bacc as bacc
import concourse.mybir as mybir
import concourse.tile as tile

mdt = mybir.dt
AF = mybir.ActivationFunctionType
ALU = mybir.AluOpType

NCORES = 8
EPS = 1e-5
RUNA = 12  # next-layer MLP runahead blocks emitted before the stats-AllReduce
RUNB = 12  # additional runahead blocks emitted before the AllGather


# ---------------------------------------------------------------- host prep


def _balance_windows(deg, nwin, cap_n=128):
    """Assign local nodes to nwin windows, balancing edge load (greedy LPT)."""
    order = np.argsort(-deg, kind="stable")
    wload = np.zeros(nwin, dtype=np.int64)
    wn = np.zeros(nwin, dtype=np.int64)
    assign = np.full(deg.shape[0], -1, dtype=np.int64)
    for v in order:
        cand = np.where(wn < cap_n)[0]
        w = cand[np.argmin(wload[cand])]
        assign[v] = w
        wload[w] += deg[v]
        wn[w] += 1
    return assign, wload, wn


def prep_inputs(inputs):
    x = np.asarray(inputs["x"], np.float32)
    edge_attr = np.asarray(inputs["edge_attr"], np.float32)
    edge_index = np.asarray(inputs["edge_index"])
    N, IN = x.shape
    E, EA = edge_attr.shape
    H = np.asarray(inputs["init_W"]).shape[1]
    L = np.asarray(inputs["edge_mlp_W"]).shape[0]
    T = np.asarray(inputs["final_W"]).shape[1]
    shard = N // NCORES
    nwin = (shard + 127) // 128
    shard_pad = nwin * 128
    src = edge_index[0].astype(np.int64)
    dst = edge_index[1].astype(np.int64)
    owner = dst // shard
    dst_local = dst - owner * shard

    # per-core window assignment (node permutation)
    assigns, wns = [], []
    pos_of_node = np.empty((NCORES, shard), np.int64)  # node -> padded position
    node_at_pos = np.full((NCORES, shard_pad), -1, np.int64)
    cnt = np.zeros((NCORES, nwin), np.int64)
    for c in range(NCORES):
        dl = dst_local[owner == c]
        deg = np.bincount(dl, minlength=shard)
        assign, wload, wn = _balance_windows(deg, nwin)
        assigns.append(assign)
        wns.append(wn)
        cnt[c] = wload
        row_next = np.zeros(nwin, np.int64)
        for v in range(shard):
            w = assign[v]
            p = w * 128 + row_next[w]
            row_next[w] += 1
            pos_of_node[c, v] = p
            node_at_pos[c, p] = v

    bw = np.maximum((cnt + 127) // 128, 1).max(axis=0)  # blocks per window
    block_win = []
    for w in range(nwin):
        block_win += [w] * int(bw[w])
    B = len(block_win)

    # slot assignment: edges of (core, window) packed into that window's blocks
    eslot = np.full((NCORES, B, 128), -1, dtype=np.int64)
    wb0 = np.concatenate([[0], np.cumsum(bw)])  # first block of window w
    for c in range(NCORES):
        ec = np.where(owner == c)[0]
        wids = assigns[c][dst_local[ec]]
        for w in range(nwin):
            es = ec[wids == w]
            b0 = wb0[w]
            for j, e in enumerate(es):
                eslot[c, b0 + j // 128, j % 128] = e

    # host-side h0 (layer-0 table): h0 = x @ init_W + init_b
    h0 = x @ np.asarray(inputs["init_W"], np.float32) + np.asarray(
        inputs["init_b"], np.float32
    )

    # o-major reorder of edge MLP weights: col j = i*H + o -> o*H + i
    Wm = np.asarray(inputs["edge_mlp_W"], np.float32).reshape(L, EA, H, H)
    Wm = Wm.transpose(0, 1, 3, 2).reshape(L, EA, H * H)
    bm = np.asarray(inputs["edge_mlp_b"], np.float32).reshape(L, H, H)
    bm = bm.transpose(0, 2, 1).reshape(L, H * H)
    W_aug = np.concatenate([Wm, bm[:, None, :]], axis=1)  # [L, EA+1, H*H]

    rootW = np.asarray(inputs["root_W"], np.float32)
    root_aug = np.zeros((L, 2 * H + 1, H), np.float32)
    root_aug[:, :H, :] = rootW
    root_aug[:, 2 * H, :] = np.asarray(inputs["root_b"], np.float32)

    trans_aug = np.concatenate(
        [np.asarray(inputs["trans_W"], np.float32),
         np.asarray(inputs["trans_b"], np.float32)[:, None, :]], axis=1
    )  # [L, 2H+1, H]

    final_aug = np.zeros((2 * H + 1, T), np.float32)
    final_aug[:H, :] = np.asarray(inputs["final_W"], np.float32)
    final_aug[2 * H, :] = np.asarray(inputs["final_b"], np.float32)

    gn = np.concatenate(
        [np.asarray(inputs["gn_w"], np.float32),
         np.asarray(inputs["gn_b"], np.float32),
         np.asarray(inputs["gn_ms"], np.float32)], axis=1
    )[:, None, :]  # [L, 1, 3H]
    fgn = np.concatenate(
        [np.asarray(inputs["fgn_w"], np.float32),
         np.asarray(inputs["fgn_b"], np.float32),
         np.asarray(inputs["fgn_ms"], np.float32)], axis=0
    )[None, :]  # [1, 3T]

    iota = np.broadcast_to(np.arange(128, dtype=np.float32), (128, 128))
    ident = np.eye(128, dtype=np.float32)
    ones_row = np.ones((1, 128), np.float32)

    in_maps = []
    for c in range(NCORES):
        es = eslot[c]
        valid = es >= 0
        esc = np.where(valid, es, 0)
        flat = esc.reshape(-1)
        vflat = valid.reshape(-1)

        attrT_aug = np.zeros((EA + 1, B * 128), np.float32)
        attrT_aug[:EA, :] = edge_attr[flat].T * vflat
        attrT_aug[EA, :] = vflat.astype(np.float32)

        sg = src[flat]
        gidx = (sg // shard) * shard_pad + pos_of_node[sg // shard, sg % shard]
        gidx = np.where(vflat, gidx, 0).astype(np.int32)
        src_gidx = gidx.reshape(B, 128).T.copy()

        # pre-gathered layer-0 h[src] (pad slots zeroed)
        hsrc0 = (h0[sg] * vflat[:, None]).reshape(B, 128, H).transpose(1, 0, 2)

        wl = assigns[c][dst_local[flat]]
        dr = np.where(vflat,
                      pos_of_node[c, dst_local[flat]] - wl * 128, -1.0)
        dst_rel = dr.astype(np.float32).reshape(B, 128).T
        # host-built one-hot scatter blocks [128 slot, B, 128 dstrow]
        onehot = (np.arange(128, dtype=np.float32)[None, None, :]
                  == dst_rel[:, :, None])

        # own shard h0 in permuted layout [128, nwin, H]
        h0_own = np.zeros((shard_pad, H), np.float32)
        vmask = node_at_pos[c] >= 0
        h0_own[vmask] = h0[c * shard + node_at_pos[c, vmask]]
        h0_own = h0_own.reshape(nwin, 128, H).transpose(1, 0, 2)

        mask = (np.arange(128)[:, None] < wns[c][None, :]).astype(np.float32)

        in_maps.append(
            {
                "attrT_aug": np.ascontiguousarray(attrT_aug).astype(ml_dtypes.bfloat16),
                "src_gidx": np.ascontiguousarray(src_gidx),
                "onehot": np.ascontiguousarray(onehot).astype(ml_dtypes.bfloat16),
                "hsrc0": np.ascontiguousarray(hsrc0).astype(ml_dtypes.bfloat16),
                "h0_own": np.ascontiguousarray(h0_own).astype(ml_dtypes.bfloat16),
                "mask": np.ascontiguousarray(mask),
                "W_aug": W_aug.astype(ml_dtypes.bfloat16),
                "root_aug": root_aug.astype(ml_dtypes.bfloat16),
                "trans_aug": trans_aug.astype(ml_dtypes.bfloat16),
                "final_aug": final_aug.astype(ml_dtypes.bfloat16),
                "gn": gn,
                "fgn": fgn,
                "ident": ident.astype(ml_dtypes.bfloat16),
                "ones_row": ones_row,
            }
        )

    shapes = dict(
        N=N, E=E, IN=IN, H=H, EA=EA, T=T, L=L, shard=shard,
        shard_pad=shard_pad, nub=nwin, B=B, block_win=tuple(block_win),
        bw=tuple(int(v) for v in bw), nwin=nwin,
    )
    perms = node_at_pos  # for output unpermute
    return in_maps, shapes, perms


# ------------------------------------------------------------- device build


def build_program(s):
    H, EA, T, L = s["H"], s["EA"], s["T"], s["L"]
    B, nub, nwin = s["B"], s["nub"], s["nwin"]
    shard_pad = s["shard_pad"]
    block_win = s["block_win"]
    bw = s["bw"]
    HH = H * H
    HHH = HH // 2
    n_total = shard_pad * NCORES
    n_real = s["N"]

    nc = bacc.Bacc("TRN2", target_bir_lowering=False, debug=False,
                   enable_asserts=False, num_devices=NCORES)

    def din(name, shape, dtype=mdt.float32):
        return nc.dram_tensor(name, shape, dtype, kind="ExternalInput").ap()

    attrT = din("attrT_aug", [EA + 1, B * 128], mdt.bfloat16)
    src_gidx = din("src_gidx", [128, B], mdt.int32)
    onehot_in = din("onehot", [128, B, 128], mdt.bfloat16)
    hsrc0_in = din("hsrc0", [128, B, H], mdt.bfloat16)
    h0_own_in = din("h0_own", [128, nub, H], mdt.bfloat16)
    mask_in = din("mask", [128, nub])
    W_in = din("W_aug", [L, EA + 1, HH], mdt.bfloat16)
    root_in = din("root_aug", [L, 2 * H + 1, H], mdt.bfloat16)
    trans_in = din("trans_aug", [L, 2 * H + 1, H], mdt.bfloat16)
    final_in = din("final_aug", [2 * H + 1, T], mdt.bfloat16)
    gn_in = din("gn", [L, 1, 3 * H])
    fgn_in = din("fgn", [1, 3 * T])
    ident_in = din("ident", [128, 128], mdt.bfloat16)
    ones_in = din("ones_row", [1, 128])

    out_dram = nc.dram_tensor("out", [shard_pad, T], mdt.float32,
                              kind="ExternalOutput").ap()

    rg = [list(range(NCORES))]

    with tile.TileContext(nc) as tc:
        with (
            tc.tile_pool(name="const", bufs=1) as cpool,
            tc.tile_pool(name="hbuf", bufs=1) as hpool,
            tc.tile_pool(name="ew", bufs=(RUNA + RUNB) // 2 + 2) as ewpool,
            tc.tile_pool(name="tmp", bufs=4) as tmppool,
            tc.tile_pool(name="rows", bufs=10) as rpool,
            tc.tile_pool(name="ps", bufs=1, space="PSUM") as ps,
            tc.tile_pool(name="dram", bufs=1, space="DRAM") as dram,
        ):
            def load(pool, shape, ap, dtype=mdt.float32, tag=None):
                t = pool.tile(shape, dtype, tag=tag)
                nc.sync.dma_start(t[:], ap)
                return t

            # startup DMAs: first-needed chunks on the SP queue, bulk spread
            # across the Act/DVE queues so compute starts in a few us
            SPL = RUNA * 128
            attrT_sb = cpool.tile([EA + 1, B * 128], mdt.bfloat16, tag="attrT")
            nc.sync.dma_start(attrT_sb[:, 0:SPL], attrT[:, 0:SPL])
            W_l = [cpool.tile([EA + 1, HH], mdt.bfloat16, tag=f"W{li}",
                              name=f"W_{li}") for li in range(L)]
            nc.sync.dma_start(W_l[0][:], W_in[0])
            hsrc0_sb = cpool.tile([128, B, H], mdt.bfloat16, tag="hsrc0")
            nc.sync.dma_start(hsrc0_sb[:, 0:RUNA, :], hsrc0_in[:, 0:RUNA, :])
            onehot_sb = cpool.tile([128, B, 128], mdt.bfloat16, tag="onehot")
            nc.sync.dma_start(onehot_sb[:, 0:RUNA, :], onehot_in[:, 0:RUNA, :])
            nc.scalar.dma_start(attrT_sb[:, SPL:], attrT[:, SPL:])
            nc.scalar.dma_start(onehot_sb[:, RUNA:, :], onehot_in[:, RUNA:, :])
            nc.gpsimd.dma_start(hsrc0_sb[:, RUNA:, :], hsrc0_in[:, RUNA:, :])
            for li in range(1, L):
                nc.gpsimd.dma_start(W_l[li][:], W_in[li])
            idx_sb = cpool.tile([128, B], mdt.int32, tag="sidx")
            nc.gpsimd.dma_start(idx_sb[:], src_gidx[:])
            mask_sb = cpool.tile([128, nub], mdt.float32, tag="mask")
            nc.gpsimd.dma_start(mask_sb[:], mask_in[:])
            final_sb = cpool.tile([2 * H + 1, T], mdt.bfloat16, tag="finalw")
            nc.gpsimd.dma_start(final_sb[:], final_in[:])
            ident_sb = load(cpool, [128, 128], ident_in[:], mdt.bfloat16,
                            tag="ident")
            onesr_sb = load(cpool, [1, 128], ones_in[:], tag="onesr")
            fgn_sb = load(cpool, [1, 3 * T], fgn_in[:], tag="fgn")
            root_l = [load(cpool, [2 * H + 1, H], root_in[li], mdt.bfloat16,
                           tag=f"rw{li}") for li in range(L)]
            trans_l = [load(cpool, [2 * H + 1, H], trans_in[li], mdt.bfloat16,
                            tag=f"tw{li}") for li in range(L)]
            gn_l = [load(cpool, [1, 3 * H], gn_in[li], tag=f"gn{li}")
                    for li in range(L)]

            # ---- persistent tiles
            hA = hpool.tile([128, nub, H], mdt.bfloat16)
            hB = hpool.tile([128, nub, H], mdt.bfloat16)
            hc_all = hpool.tile([128, nub, H], mdt.bfloat16)
            t1_all = hpool.tile([128, nub, H], mdt.float32)
            conv_sb = hpool.tile([128, nub, H], mdt.float32)
            st_sb = hpool.tile([128, nub, 2 * H], mdt.float32)
            fst_sb = hpool.tile([128, nub, 2 * T], mdt.float32)
            hsrc_sb = hpool.tile([128, B, H], mdt.bfloat16)
            catT_all = hpool.tile([2 * H + 1, nub, 128], mdt.bfloat16)
            fo_sb = hpool.tile([128, nub, T], mdt.float32)
            y_sb = hpool.tile([128, nub, T], mdt.float32)
            cd_sb = hpool.tile([128, 2 * H], mdt.float32)
            fcd_sb = hpool.tile([128, 2 * T], mdt.float32)
            stats_sb = hpool.tile([1, 2 * H], mdt.float32)
            fstats_sb = hpool.tile([1, 2 * T], mdt.float32)

            nc.vector.memset(catT_all[H : 2 * H, :, :], 0.0)
            nc.vector.memset(catT_all[2 * H : 2 * H + 1, :, :], 1.0)
            nc.sync.dma_start(hA[:], h0_own_in[:])

            # warmup collective: absorbs first-collective latency during
            # the layer-0 edge phase (result unused)
            warm_in = dram.tile([1, 8], mdt.float32)
            warm_out = dram.tile([1, 8], mdt.float32, addr_space="Shared")
            wrow = rpool.tile([1, 8], mdt.float32, tag="warm")
            nc.vector.memset(wrow[:], 0.0)
            nc.sync.dma_start(warm_in[:], wrow[:])
            nc.gpsimd.collective_compute(
                "AllReduce", ALU.add, replica_groups=rg,
                ins=[warm_in.opt()], outs=[warm_out.opt()],
            )

            hstage_dram = dram.tile([shard_pad, H], mdt.bfloat16)
            htable_l = [None] + [
                dram.tile([n_total, H], mdt.bfloat16, addr_space="Shared",
                          tag=f"htable{li}", name=f"htable{li}")
                for li in range(1, L)
            ]
            st_in = dram.tile([1, 2 * H], mdt.float32)
            st_out_l = [dram.tile([1, 2 * H], mdt.float32, addr_space="Shared",
                                  tag=f"stout{li}", name=f"stout{li}")
                        for li in range(L)]
            fst_in = dram.tile([1, 2 * T], mdt.float32)
            fst_out = dram.tile([1, 2 * T], mdt.float32, addr_space="Shared")

            hstage_v = hstage_dram[:].rearrange("(u p) f -> p u f", p=128)

            def rstd_row(dstrow, varrow, width, tag):
                """dstrow = 1/sqrt(varrow+EPS) via reciprocal+sqrt+Newton."""
                ve = rpool.tile([1, width], mdt.float32, tag=tag)
                nc.vector.tensor_scalar_add(ve[:], varrow, EPS)
                r2 = rpool.tile([1, width], mdt.float32, tag=tag)
                nc.vector.reciprocal(r2[:], ve[:])
                r0 = rpool.tile([1, width], mdt.float32, tag=tag)
                nc.scalar.activation(r0[:], r2[:], AF.Sqrt)
                t0 = rpool.tile([1, width], mdt.float32, tag=tag)
                nc.vector.tensor_mul(t0[:], r0[:], r0[:])
                nc.vector.tensor_mul(t0[:], t0[:], ve[:])
                nc.vector.scalar_tensor_tensor(
                    t0[:], t0[:], -0.5, r0[:], op0=ALU.mult, op1=ALU.mult
                )
                nc.vector.scalar_tensor_tensor(
                    dstrow, r0[:], 1.5, t0[:], op0=ALU.mult, op1=ALU.add
                )

            def cd_rows(crow, srow, gnw, gnb, gnms, width, tag):
                """crow[0:w] = C = rstd*w ; crow[w:2w] = D = b - ms*mean*C."""
                mean = rpool.tile([1, width], mdt.float32, tag=tag)
                nc.vector.tensor_scalar_mul(mean[:], srow[:, width : 2 * width],
                                            1.0 / n_real)
                msq = rpool.tile([1, width], mdt.float32, tag=tag)
                nc.vector.tensor_scalar_mul(msq[:], srow[:, 0:width],
                                            1.0 / n_real)
                mm = rpool.tile([1, width], mdt.float32, tag=tag)
                nc.vector.tensor_mul(mm[:], mean[:], mean[:])
                nc.vector.tensor_mul(mm[:], mm[:], gnms)
                co = rpool.tile([1, width], mdt.float32, tag=tag)
                nc.vector.tensor_scalar(co[:], gnms, -1.0, 2.0, op0=ALU.mult,
                                        op1=ALU.add)
                nc.vector.tensor_mul(mm[:], mm[:], co[:])
                var = rpool.tile([1, width], mdt.float32, tag=tag)
                nc.vector.tensor_sub(var[:], msq[:], mm[:])
                rstd = rpool.tile([1, width], mdt.float32, tag=tag)
                rstd_row(rstd[:], var[:], width, tag)
                nc.vector.tensor_mul(crow[:, 0:width], rstd[:], gnw)
                nc.vector.tensor_mul(crow[:, width : 2 * width], mean[:], gnms)
                nc.vector.tensor_mul(crow[:, width : 2 * width],
                                     crow[:, width : 2 * width],
                                     crow[:, 0:width])
                nc.vector.scalar_tensor_tensor(
                    crow[:, width : 2 * width], crow[:, width : 2 * width],
                    -1.0, gnb, op0=ALU.mult, op1=ALU.add,
                )

            def emit_mlp_relu2(li, bp):
                """MLP + Act relu for block pair (2bp, 2bp+1) -> one bf16
                double-wide SBUF ring tile (enables paired DVE ops)."""
                ew2 = ewpool.tile([128, 2, HH], mdt.bfloat16, tag="ew")
                for j in (0, 1):
                    b = 2 * bp + j
                    pre = ps.tile([128, HH], mdt.float32, tag="pre", bufs=3)
                    a_sl = attrT_sb[:, b * 128 : (b + 1) * 128]
                    nc.tensor.matmul(pre[:, 0:HHH], a_sl, W_l[li][:, 0:HHH],
                                     start=True, stop=True)
                    nc.tensor.matmul(pre[:, HHH:HH], a_sl, W_l[li][:, HHH:HH],
                                     start=True, stop=True)
                    nc.scalar.activation(ew2[:, j, :], pre[:], AF.Relu)
                return ew2

            pending = {}
            hcur, hnxt = hA, hB

            for li in range(L):
                hsrc_v = hsrc0_sb if li == 0 else hsrc_sb
                if li > 0:
                    htable = htable_l[li]
                    for b in range(B):
                        nc.gpsimd.indirect_dma_start(
                            out=hsrc_sb[:, b, :],
                            out_offset=None,
                            in_=htable[:],
                            in_offset=bass.IndirectOffsetOnAxis(
                                ap=idx_sb[:, b : b + 1], axis=0
                            ),
                        )

                # ---- edge phase: paired per-block i-reduce (bf16 2x) then
                # narrow 32-col one-hot scatters into a tiny PSUM accumulator
                assert all(v % 2 == 0 for v in bw)
                b = 0
                for w in range(nwin):
                    aggI = ps.tile([128, H], mdt.float32, tag="agg", bufs=2)
                    for half in range(bw[w] // 2):
                        bp = b // 2
                        if (li, bp) in pending:
                            ew2 = pending.pop((li, bp))
                        else:
                            ew2 = emit_mlp_relu2(li, bp)
                        h_bc = (hsrc_v[:, b : b + 2, :].unsqueeze(2)
                                .broadcast_to([128, 2, H, H]))
                        tmp2 = tmppool.tile([128, 2, HH], mdt.bfloat16,
                                            tag="tmp")
                        tmp_v = tmp2[:].rearrange("p t (o i) -> p t o i",
                                                  o=H, i=H)
                        nc.vector.tensor_tensor(
                            tmp_v,
                            ew2[:].rearrange("p t (o i) -> p t o i", o=H,
                                             i=H),
                            h_bc, op=ALU.mult)
                        msg2 = tmppool.tile([128, 2, H], mdt.bfloat16,
                                            tag="msg")
                        with nc.allow_low_precision(reason="msg bf16 i-sum"):
                            nc.vector.tensor_reduce(
                                msg2[:], tmp_v, axis=mybir.AxisListType.X,
                                op=ALU.add)
                        for j in (0, 1):
                            nc.tensor.matmul(
                                aggI[:], onehot_sb[:, b + j, :],
                                msg2[:, j, :],
                                start=(half == 0 and j == 0),
                                stop=(half == bw[w] // 2 - 1 and j == 1))
                        b += 2
                    nc.scalar.activation(conv_sb[:, w, :], aggI[:], AF.Copy)

                # ---- node pass 1 (deferred, stage-major pipelined)
                for w in range(nwin):
                    tp = ps.tile([H, 128], mdt.bfloat16, tag="agg", bufs=2)
                    nc.tensor.transpose(tp[:], hcur[:, w, :], ident_sb[:])
                    if w % 2 == 0:
                        nc.scalar.activation(catT_all[0:H, w, :], tp[:],
                                             AF.Copy)
                    else:
                        nc.vector.tensor_copy(catT_all[0:H, w, :], tp[:])
                rt_all = ps.tile([128, nub * H], mdt.float32, tag="pre",
                                 bufs=3)
                for w in range(nwin):
                    nc.tensor.matmul(rt_all[:, w * H : (w + 1) * H],
                                     catT_all[:, w, :], root_l[li][:],
                                     start=True, stop=True)
                nc.vector.tensor_add(
                    conv_sb[:].rearrange("p u f -> p (u f)"),
                    conv_sb[:].rearrange("p u f -> p (u f)"), rt_all[:])
                nc.vector.tensor_mul(st_sb[:, :, 0:H], conv_sb[:],
                                     conv_sb[:])
                nc.vector.tensor_copy(st_sb[:, :, H : 2 * H], conv_sb[:])
                smm = ps.tile([1, 2 * H], mdt.float32, tag="pre", bufs=3)
                for w in range(nwin):
                    nc.tensor.matmul(smm[:], mask_sb[:, w : w + 1],
                                     st_sb[:, w, :], start=(w == 0),
                                     stop=(w == nwin - 1))
                nc.scalar.activation(stats_sb[:], smm[:], AF.Copy)

                # ---- stats AllReduce
                nc.sync.dma_start(st_in[:], stats_sb[:])
                st_out = st_out_l[li]
                nc.gpsimd.collective_compute(
                    "AllReduce", ALU.add, replica_groups=rg,
                    ins=[st_in.opt()], outs=[st_out.opt()],
                )

                # ---- next-layer MLP runahead (fills the AllReduce window)
                if li + 1 < L:
                    for rp in range(RUNA // 2):
                        pending[(li + 1, rp)] = emit_mlp_relu2(li + 1, rp)

                srow2 = rpool.tile([1, 2 * H], mdt.float32, tag="srow")
                nc.sync.dma_start(srow2[:], st_out[:])

                # ---- C/D rows + broadcast
                crow = rpool.tile([1, 2 * H], mdt.float32, tag="cdrow")
                cd_rows(crow, srow2, gn_l[li][:, 0:H], gn_l[li][:, H : 2 * H],
                        gn_l[li][:, 2 * H : 3 * H], H, "nrow")
                cd_ps = ps.tile([128, 2 * H], mdt.float32, tag="pre", bufs=3)
                nc.tensor.matmul(cd_ps[:], onesr_sb[:], crow[:], start=True,
                                 stop=True)
                nc.scalar.activation(cd_sb[:], cd_ps[:], AF.Copy)

                # ---- node pass 2 (batched)
                nc.vector.tensor_tensor(
                    t1_all[:], conv_sb[:],
                    cd_sb[:, 0:H].unsqueeze(1).broadcast_to([128, nub, H]),
                    op=ALU.mult)
                nc.vector.tensor_tensor(
                    t1_all[:], t1_all[:],
                    cd_sb[:, H : 2 * H].unsqueeze(1)
                    .broadcast_to([128, nub, H]),
                    op=ALU.add)
                nc.vector.scalar_tensor_tensor(
                    hc_all[:], t1_all[:], 0.0, hcur[:],
                    op0=ALU.max, op1=ALU.add)
                for u in range(nub):
                    tp2 = ps.tile([H, 128], mdt.bfloat16, tag="agg", bufs=2)
                    nc.tensor.transpose(tp2[:], hc_all[:, u, :], ident_sb[:])
                    if u % 2 == 0:
                        nc.scalar.activation(catT_all[H : 2 * H, u, :],
                                             tp2[:], AF.Copy)
                    else:
                        nc.vector.tensor_copy(catT_all[H : 2 * H, u, :],
                                              tp2[:])
                for u in range(nub):
                    tr = ps.tile([128, H], mdt.float32, tag="pre", bufs=3)
                    nc.tensor.matmul(tr[:], catT_all[:, u, :], trans_l[li][:],
                                     start=True, stop=True)
                    if u % 2 == 0:
                        nc.scalar.activation(hnxt[:, u, :], tr[:], AF.Relu)
                    else:
                        nc.vector.tensor_scalar_max(hnxt[:, u, :], tr[:], 0.0)

                # runahead-B: fills the AllGather window (PE queue would
                # otherwise head-block on the next layer's first scatter)
                if li + 1 < L:
                    for rp in range(RUNA // 2, (RUNA + RUNB) // 2):
                        pending[(li + 1, rp)] = emit_mlp_relu2(li + 1, rp)
                    nc.sync.dma_start(hstage_v, hnxt[:])
                    nc.gpsimd.collective_compute(
                        "AllGather", ALU.bypass, replica_groups=rg,
                        ins=[hstage_dram.opt()], outs=[htable_l[li + 1].opt()],
                    )

                hcur, hnxt = hnxt, hcur

            # ============ final (fused, stage-major) ============
            for u in range(nub):
                tp3 = ps.tile([H, 128], mdt.bfloat16, tag="agg", bufs=2)
                nc.tensor.transpose(tp3[:], hcur[:, u, :], ident_sb[:])
                if u % 2 == 0:
                    nc.scalar.activation(catT_all[0:H, u, :], tp3[:], AF.Copy)
                else:
                    nc.vector.tensor_copy(catT_all[0:H, u, :], tp3[:])
            for u in range(nub):
                f_ps = ps.tile([128, T], mdt.float32, tag="pre", bufs=3)
                nc.tensor.matmul(f_ps[:], catT_all[:, u, :], final_sb[:],
                                 start=True, stop=True)
                if u % 2 == 0:
                    nc.scalar.activation(fo_sb[:, u, :], f_ps[:], AF.Copy)
                else:
                    nc.vector.tensor_copy(fo_sb[:, u, :], f_ps[:])
            nc.vector.tensor_mul(fst_sb[:, :, 0:T], fo_sb[:], fo_sb[:])
            nc.vector.tensor_copy(fst_sb[:, :, T : 2 * T], fo_sb[:])
            fsmm = ps.tile([1, 2 * T], mdt.float32, tag="agg", bufs=2)
            for u in range(nub):
                nc.tensor.matmul(fsmm[:], mask_sb[:, u : u + 1],
                                 fst_sb[:, u, :], start=(u == 0),
                                 stop=(u == nub - 1))
            nc.scalar.activation(fstats_sb[:], fsmm[:], AF.Copy)

            nc.sync.dma_start(fst_in[:], fstats_sb[:])
            nc.gpsimd.collective_compute(
                "AllReduce", ALU.add, replica_groups=rg,
                ins=[fst_in.opt()], outs=[fst_out.opt()],
            )
            fsrow2 = rpool.tile([1, 2 * T], mdt.float32, tag="fsrow")
            nc.sync.dma_start(fsrow2[:], fst_out[:])

            fcrow = rpool.tile([1, 2 * T], mdt.float32, tag="fcdrow")
            cd_rows(fcrow, fsrow2, fgn_sb[:, 0:T], fgn_sb[:, T : 2 * T],
                    fgn_sb[:, 2 * T : 3 * T], T, "frow")
            fcd_ps = ps.tile([128, 2 * T], mdt.float32, tag="pre", bufs=3)
            nc.tensor.matmul(fcd_ps[:], onesr_sb[:], fcrow[:], start=True,
                             stop=True)
            nc.scalar.activation(fcd_sb[:], fcd_ps[:], AF.Copy)

            nc.vector.tensor_tensor(
                y_sb[:], fo_sb[:],
                fcd_sb[:, 0:T].unsqueeze(1).broadcast_to([128, nub, T]),
                op=ALU.mult)
            nc.vector.tensor_tensor(
                y_sb[:], y_sb[:],
                fcd_sb[:, T : 2 * T].unsqueeze(1).broadcast_to([128, nub, T]),
                op=ALU.add)
            nc.vector.tensor_scalar_max(y_sb[:], y_sb[:], 0.0)
            out_v = out_dram.rearrange("(u p) f -> p u f", p=128)
            nc.sync.dma_start(out_v, y_sb[:])

    nc.compile()
    return nc


# ------------------------------------------------------------------ driver

_CACHE = {}


def kernel(**inputs) -> np.ndarray:
    in_maps, s, node_at_pos = prep_inputs(inputs)
    key = (s["N"], s["E"], s["B"], s["block_win"])
    if key not in _CACHE:
        _CACHE[key] = build_program(s)
    nc = _CACHE[key]

    from concourse.bass_utils import run_bass_kernel_spmd

    res = run_bass_kernel_spmd(nc, in_maps, core_ids=list(range(NCORES)))
    shard, T, N = s["shard"], s["T"], s["N"]
    out = np.empty((N, T), np.float32)
    for c in range(NCORES):
        rows = res.results[c]["out"]
        vmask = node_at_pos[c] >= 0
        out[c * shard + node_at_pos[c, vmask]] = rows[vmask]
    return out.astype(np.float32)
